# revision 1
# baseline (speedup 1.0000x reference)
"""Trainium2 Bass kernel for nn_NeuronS3DiffUpsample2D.

Reference computation (per sample b):
    up   = nearest-2x-upsample(x[b])                       # [C, 320, 320]
    w    = Wb + 0.25 * einsum('or,rikl->oikl', lora_up, lora_down)
    w_b  = w * de_mod[b, None, :, None, None]              # modulate input chans
    dem  = rsqrt(sum_{i,k,l} w_b^2 + eps)                  # per output chan
    y[b] = conv2d(up, w_b * dem, SAME) + bias

Key algebraic transform: a 3x3 SAME conv on a 2x nearest-upsampled image
decomposes into 4 output phases (di, dj in {0,1}), each a 2x2 conv on the
ORIGINAL 160x160 input:
    y[2i+di, 2j+dj] = sum_{a,b in {0,1}} K[di,dj,a,b] @ x[i+a+di-1, j+b+dj-1]
where the 16 [O, I] matrices K are sums of 1/2/4 of the 9 taps of w.
This is 4/9 of the naive FLOPs and never materializes the upsampled image.

Since the demod scale is per output channel and conv is linear in w, the conv
OUTPUT is scaled by dem[o] (per-partition scalar) at PSUM eviction, fused with
the bias add; weights are only modulated by de_mod on the input-channel axis.

Sharding: data-parallel over batch B=8 across 8 NeuronCores; each core builds
its own per-sample weights locally (replicated W/lora are tiny).  Host-side
work is layout only: per-sample slicing, weight transposition, fp32->f32r
rounding of x.  All arithmetic (lora matmul, modulation, demod, conv) is on
device.

Matmuls use float32r (fp32 rounded to 8-bit exp / 11-bit mantissa; PE runs it
at 1 cycle/row for moving free dim >= 256 - same speed as bf16 with 8x better
precision).  The input is banded into 6 SBUF tiles so conv matmuls start as
soon as the first band lands instead of waiting for the full 13 MB input.
"""

import sys
import numpy as np
from contextlib import ExitStack

try:
    import concourse.bass as bass
except ImportError:  # grading env without the axon PYTHONPATH
    sys.path.insert(0, "/opt/trn_rl_repo")
    import concourse.bass as bass
import concourse.tile as tile
from concourse import bacc, mybir
from concourse.bass_utils import run_bass_kernel_spmd

B, C, H, W = 8, 128, 160, 160
RANK = 32
SCALING = 0.25
EPS = 1e-8
WP = W + 2          # padded row length (zero col on each side)
R_BLK = 3           # x-rows per matmul block -> N = 3*160 = 480 <= 512
BAND_BLOCKS = 9     # blocks per input band
BAND_ROWS = BAND_BLOCKS * R_BLK      # 27 x-rows per band
NBANDS = (H + BAND_ROWS - 1) // BAND_ROWS   # 6
BAND_TROWS = BAND_ROWS + 2           # tile rows incl. halo (29)
NCORES = 8

f32 = mybir.dt.float32
f32r = mybir.dt.float32r


def _conv_kernel(ctx, tc, y, x, dmbias, wbT, luT, ldT, ident2):
    nc = tc.nc
    AF = mybir.ActivationFunctionType
    ALU = mybir.AluOpType
    AX = mybir.AxisListType

    const = ctx.enter_context(tc.tile_pool(name="const", bufs=1))
    bands = ctx.enter_context(tc.tile_pool(name="bands", bufs=3))

    comb = const.tile([128, 16, C], f32r)        # 16 combined taps, [i, slot, o]
    demP = const.tile([128, 1], f32)             # rsqrt demod, per output chan
    dmb = const.tile([128, 3], f32)              # de_mod[i], bias[o], 0.25*de_mod
    zrow = const.tile([128, WP], f32)

    # de_mod/bias arrive as a [2,128] row pair (single-descriptor DMA; a
    # [128,1] DMA is 128 4-byte descriptors and clogs the queue) and are
    # PE-transposed onto partitions.
    dmbR = const.tile([2, C], f32)
    nc.sync.dma_start(dmbR[:], dmbias[:])
    id2 = const.tile([2, 2], f32)
    nc.sync.dma_start(id2[:], ident2[:])

    nc.vector.memset(zrow[:], 0.0)

    # ---- input bands: x rows [27b-1, 27b+27] in tile rows [0, 28];
    # borders zeroed via DVE f32->f32r copies, data DMA'd on the ACT ring.
    band_tiles = []
    for bb in range(NBANDS):
        lo = BAND_ROWS * bb - 1
        hi = min(BAND_ROWS * bb + BAND_ROWS, H)
        nrows = hi - lo + 1
        bt = bands.tile([128, BAND_TROWS, WP], f32r, tag="band", name=f"band{bb}")
        r0, r1 = max(0, lo), min(H - 1, hi)          # real x rows
        # SWDGE via the otherwise-idle GpSimd engine: HWDGE descriptor
        # generation for these many-descriptor DMAs would occupy the
        # sync/ACT sequencer for ~10us and stall evictions behind it.
        nc.gpsimd.dma_start(
            bt[:, r0 - lo : r1 - lo + 1, 1 : 1 + W], x[:, r0 : r1 + 1, :]
        )
        nc.vector.tensor_copy(bt[:, 0:nrows, 0], zrow[:, 0:nrows])
        nc.vector.tensor_copy(bt[:, 0:nrows, WP - 1], zrow[:, 0:nrows])
        if lo < 0:
            nc.vector.tensor_copy(bt[:, 0, :], zrow[:])
        if hi >= H:
            nc.vector.tensor_copy(bt[:, hi - lo, :], zrow[:])
        band_tiles.append((bt, lo, nrows))

    # ---- weight stage ----
    with tc.tile_pool(name="wtmp", bufs=1) as wtmp, tc.tile_pool(
        name="wpsum", bufs=1, space="PSUM"
    ) as wpsum:
        # smallest tensors first: the delta matmuls need only LUTn + LD9
        LUTn = wtmp.tile([RANK, C], f32)         # lora_up^T: [r, o]
        nc.sync.dma_start(LUTn[:], luT[:])
        LD9 = wtmp.tile([RANK, 9, C], f32)       # lora_down^T: [r, t, i]
        nc.sync.dma_start(LD9[:], ldT[:])
        WbTS = wtmp.tile([128, 9, C], f32)       # Wb^T: [i, t, o]
        nc.sync.dma_start(WbTS[:], wbT[:])

        dmbP = wpsum.tile([128, 2], f32)
        nc.tensor.transpose(dmbP[:], dmbR[:], id2[:])
        nc.vector.tensor_copy(dmb[:, 0:2], dmbP[:])
        nc.vector.tensor_scalar_mul(dmb[:, 2:3], dmb[:, 0:1], SCALING)

        # deltaT_unscaled[i, t, o] = sum_r down[r,i,t] * up[o,r]; the 0.25
        # lora scale rides in via the fused modulation below instead of a
        # pre-scaled copy of lora_up (keeps the matmuls off the DVE chain)
        deltaP = wpsum.tile([128, 9, C], f32)
        for t in range(9):
            nc.tensor.matmul(
                deltaP[:, t, :], LD9[:, t, :], LUTn[:], start=True, stop=True
            )

        # wm3 = Wb^T*dm + deltaT*(0.25*dm); Wb^T*dm runs while the delta
        # matmuls are still in flight, the fused op is one DVE pass
        WbTm = wtmp.tile([128, 9, C], f32)
        nc.vector.tensor_scalar_mul(WbTm[:], WbTS[:], dmb[:, 0:1])
        wm3 = wtmp.tile([128, 9, C], f32)
        nc.vector.scalar_tensor_tensor(
            wm3[:], deltaP[:], dmb[:, 2:3], WbTm[:],
            op0=ALU.mult, op1=ALU.add,
        )

        # 16 combined tap matrices.  Row combos over ki (t = 3*ki + kj):
        #   (di=0, a=0): ki0        (di=0, a=1): ki1+ki2
        #   (di=1, a=0): ki0+ki1    (di=1, a=1): ki2
        # and the same pattern over kj for (dj, b).
        R01 = wtmp.tile([128, 3, C], f32)
        nc.vector.tensor_add(R01[:], wm3[:, 3:6, :], wm3[:, 6:9, :])
        R10 = wtmp.tile([128, 3, C], f32)
        nc.vector.tensor_add(R10[:], wm3[:, 0:3, :], wm3[:, 3:6, :])
        rowsrc = {
            (0, 0): wm3[:, 0:3, :],
            (0, 1): R01[:],
            (1, 0): R10[:],
            (1, 1): wm3[:, 6:9, :],
        }
        # comb slot layout: slot = 8*di + 2*a + 4*dj + b; all on DVE
        # (f32r rounds on write), phase-0 slots earliest.
        for p in range(4):
            di, dj = p >> 1, p & 1
            for q in range(4):
                a, b = q >> 1, q & 1
                S = rowsrc[(di, a)]
                dst = comb[:, 8 * di + 2 * a + 4 * dj + b, :]
                if dj == 0 and b == 0:
                    nc.vector.tensor_copy(dst, S[:, 0, :])
                elif dj == 1 and b == 1:
                    nc.vector.tensor_copy(dst, S[:, 2, :])
                elif dj == 0:
                    nc.vector.tensor_add(dst, S[:, 1, :], S[:, 2, :])
                else:
                    nc.vector.tensor_add(dst, S[:, 0, :], S[:, 1, :])

        # demod[o] = 1/sqrt(sum_{i,t} wm^2 + eps)  -- off the MM critical
        # path (only needed by the first PSUM eviction).  Square on ACT so
        # DVE can run the comb builds in parallel.
        sq3 = wtmp.tile([128, 9, C], f32)
        nc.scalar.square(sq3[:], wm3[:])
        s2 = wtmp.tile([128, C], f32)
        nc.vector.tensor_reduce(
            s2[:], sq3.rearrange("p t o -> p o t"), axis=AX.X, op=ALU.add
        )
        onesS = wtmp.tile([128, 1], f32)
        nc.vector.memset(onesS[:], 1.0)
        sP = wpsum.tile([128, 1], f32)
        nc.tensor.matmul(sP[:], s2[:], onesS[:], start=True, stop=True)
        t1 = wtmp.tile([128, 1], f32)
        nc.vector.tensor_scalar_add(t1[:], sP[:], EPS)
        t2 = wtmp.tile([128, 1], f32)
        nc.scalar.sqrt(t2[:], t1[:])
        nc.vector.reciprocal(demP[:], t2[:])

    # ---- main conv loop ----
    mpsum = ctx.enter_context(tc.tile_pool(name="mpsum", bufs=8, space="PSUM"))
    opool = ctx.enter_context(tc.tile_pool(name="obuf", bufs=3))

    for i0 in range(0, H, R_BLK):
        R = min(R_BLK, H - i0)
        bt, lo, _ = band_tiles[i0 // BAND_ROWS]
        ph = []
        for p in range(4):
            di, dj = p >> 1, p & 1
            pt = mpsum.tile([128, R * W], f32, tag="ph", name=f"ph{p}_{i0}")
            for q in range(4):
                a, b = q >> 1, q & 1
                r0 = i0 + (a + di - 1) - lo          # tile row of first x row
                co = b + dj - 1
                rhs = bt[:, r0 : r0 + R, co + 1 : co + 1 + W]
                slot = 8 * di + 2 * a + 4 * dj + b
                nc.tensor.matmul(
                    pt[:], comb[:, slot, :], rhs,
                    start=(q == 0), stop=(q == 3),
                )
            ph.append(pt)
        # interleave phases into full output rows; scale by demod, add bias
        ob = opool.tile([128, R, 2, 2 * W], f32, tag="ob", name=f"ob_{i0}")
        obv = ob.rearrange("p r d (j two) -> p r d two j", two=2)
        for p in range(4):
            di, dj = p >> 1, p & 1
            dst = obv[:, :, di, dj, :]
            srcv = ph[p].rearrange("p (r j) -> p r j", r=R)
            if dj == 0:
                nc.vector.tensor_scalar(
                    dst, srcv, demP[:, 0:1], dmb[:, 1:2],
                    op0=ALU.mult, op1=ALU.add,
                )
            else:
                nc.scalar.activation(
                    dst, srcv, AF.Identity, bias=dmb[:, 1:2], scale=demP[:, 0:1]
                )
        nc.sync.dma_start(y[:, 2 * i0 : 2 * i0 + 2 * R, :], ob[:])


def _build():
    nc = bacc.Bacc(
        "TRN2",
        target_bir_lowering=False,
        debug=False,
        enable_asserts=False,
        num_devices=NCORES,
    )
    x = nc.dram_tensor("x", [C, H, W], f32r, kind="ExternalInput").ap()
    dmbias = nc.dram_tensor("dmbias", [2, C], f32, kind="ExternalInput").ap()
    wbT = nc.dram_tensor("WbT", [C, 9 * C], f32, kind="ExternalInput").ap()
    luT = nc.dram_tensor("lora_upT", [RANK, C], f32, kind="ExternalInput").ap()
    ldT = nc.dram_tensor("lora_downT", [RANK, 9 * C], f32, kind="ExternalInput").ap()
    ident2 = nc.dram_tensor("ident2", [2, 2], f32, kind="ExternalInput").ap()
    y = nc.dram_tensor("y", [C, 2 * H, 2 * W], f32, kind="ExternalOutput").ap()

    with tile.TileContext(nc) as tc:
        with ExitStack() as ctx:
            _conv_kernel(ctx, tc, y, x, dmbias, wbT, luT, ldT, ident2)
    nc.compile()
    return nc


_CACHE = {}


def _get_nc():
    if "nc" not in _CACHE:
        _CACHE["nc"] = _build()
    return _CACHE["nc"]


def _round_f32r(a):
    """Round fp32 array to nearest float32r (8-bit exp, 11-bit mantissa;
    low 12 bits zero) - the PE's operand precision for f32r matmuls."""
    u = np.ascontiguousarray(a, dtype=np.float32).view(np.uint32).copy()
    u += 0x800
    u &= np.uint32(0xFFFFF000)
    return u.view(np.float32)


def _make_in_maps(x, de_mod, Wb, lora_up, lora_down, bias):
    x = _round_f32r(np.asarray(x, dtype=np.float32))
    de_mod = np.asarray(de_mod, dtype=np.float32)
    Wb = np.asarray(Wb, dtype=np.float32)
    lora_up = np.asarray(lora_up, dtype=np.float32)
    lora_down = np.asarray(lora_down, dtype=np.float32)
    # layout-only host prep: [O,I,3,3] -> [i, (t o)], [R,C,3,3] -> [r, (t i)]
    wbT = np.ascontiguousarray(Wb.transpose(1, 2, 3, 0).reshape(C, 9 * C))
    luT = np.ascontiguousarray(lora_up.T)
    ldT = np.ascontiguousarray(lora_down.transpose(0, 2, 3, 1).reshape(RANK, 9 * C))
    bias = np.asarray(bias, dtype=np.float32).reshape(C)
    id2 = np.eye(2, dtype=np.float32)
    in_maps = []
    for b in range(NCORES):
        in_maps.append(
            {
                "x": np.ascontiguousarray(x[b]),
                "dmbias": np.ascontiguousarray(np.stack([de_mod[b], bias])),
                "WbT": wbT,
                "lora_upT": luT,
                "lora_downT": ldT,
                "ident2": id2,
            }
        )
    return in_maps


def run(inputs, trace=False, trace_kwargs=None):
    nc = _get_nc()
    in_maps = _make_in_maps(**inputs)
    res = run_bass_kernel_spmd(
        nc,
        in_maps,
        core_ids=list(range(NCORES)),
        trace=trace,
        **(trace_kwargs or {}),
    )
    y = np.stack([res.results[b]["y"] for b in range(NCORES)], axis=0)
    return y, res


def kernel(**inputs):
    y, _ = run(inputs)
    return y



# revision 12
# speedup vs baseline: 1.0525x; 1.0525x over previous
"""Trainium2 Bass kernel for nn_NeuronS3DiffUpsample2D.

Reference computation (per sample b):
    up   = nearest-2x-upsample(x[b])                       # [C, 320, 320]
    w    = Wb + 0.25 * einsum('or,rikl->oikl', lora_up, lora_down)
    w_b  = w * de_mod[b, None, :, None, None]              # modulate input chans
    dem  = rsqrt(sum_{i,k,l} w_b^2 + eps)                  # per output chan
    y[b] = conv2d(up, w_b, SAME) * dem + bias

Key algebraic transform: a 3x3 SAME conv on a 2x nearest-upsampled image
decomposes into 4 output phases (di, dj in {0,1}), each a 2x2 conv on the
ORIGINAL 160x160 input:
    y[2i+di, 2j+dj] = sum_{a,b in {0,1}} K[di,dj,a,b] @ x[i+a+di-1, j+b+dj-1]
where the 16 [O, I] matrices K are sums of 1/2/4 of the 9 taps of w.
This is 4/9 of the naive FLOPs and never materializes the upsampled image.
The demod scale is per output channel and conv is linear in w, so the conv
OUTPUT is scaled by dem[o] at PSUM eviction, fused with the bias add.

All tensors ride bf16 (rel-err budget is 2e-2; bf16 end-to-end lands ~4e-3):
x is host-padded to [C,162,162] bf16 so every band DMA is one contiguous
descriptor per partition (no per-row 640B descriptors, no on-device border
zeroing), matmuls are bf16 (1 cycle/row at any free size - also kills the
f32r N<256 4x penalty on the R=1 tail block), and y is written bf16 and
upcast on the host.  Total HBM traffic drops from 66 MB to ~35 MB per core,
taking DMA well below the PE's ~173 us of matmul streaming.

The demod reduction runs in a second [o, (t,i)] weight layout so the
partition-axis sum becomes a free-axis reduce: 3 small early matmuls build
delta in [o,.] form, then gpsimd/DVE/ACT finish rsqrt off the PE's program
order (the baseline's ones-matmul made the PE wait ~10us on a DVE chain).
Dummy "warm" matmuls keep the PE busy through the weight stage so the
2.4 GHz pstate ramp completes before the main conv stream starts.

Sharding: data-parallel over batch B=8 across 8 NeuronCores; each core
builds its own per-sample weights locally.  Host-side work is layout only:
padding, per-sample slicing/replication, transposes, dtype casts.
"""

import os
import sys
import numpy as np
from contextlib import ExitStack

_NOWARM = bool(int(os.environ.get("K_NOWARM", "0")))
_NOGPS = bool(int(os.environ.get("K_NOGPS", "0")))
_NODEMO = bool(int(os.environ.get("K_NODEMO", "0")))   # demod=1.0 (wrong numerics, hang-test only)
_F32OUT = bool(int(os.environ.get("K_F32OUT", "0")))
_NOACTMUL = bool(int(os.environ.get("K_NOACTMUL", "0")))

try:
    import concourse.bass as bass
except ImportError:  # grading env without the axon PYTHONPATH
    sys.path.insert(0, "/opt/trn_rl_repo")
    import concourse.bass as bass
import ml_dtypes
import concourse.tile as tile
from concourse import bacc, mybir
from concourse.bass_utils import run_bass_kernel_spmd

B, C, H, W = 8, 128, 160, 160
RANK = 32
SCALING = 0.25
EPS = 1e-8
HP, WP = H + 2, W + 2      # host-padded input (zero border baked in)
R_BLK = 3                  # x-rows per matmul block -> N = 3*160 = 480 <= 512
# x-row band split: band 0 tiny so the first matmuls gate only on ~0.3 MB.
BAND_ROWS = [6, 27, 27, 27, 27, 27, 19]
BAND_START = [0, 6, 33, 60, 87, 114, 141]
NBANDS = len(BAND_ROWS)
NBLOCKS = 54               # 53 blocks of 3 rows + 1 tail row
N_WARM = 14                # PE pstate prewarm matmuls
NCORES = 8

f32 = mybir.dt.float32
bf16 = mybir.dt.bfloat16


def _conv_kernel(ctx, tc, y, x, dmb_d, dmO_d, wbT, wbO, luT, ldT):
    nc = tc.nc
    AF = mybir.ActivationFunctionType
    ALU = mybir.AluOpType
    AX = mybir.AxisListType

    const = ctx.enter_context(tc.tile_pool(name="const", bufs=1))
    bands = ctx.enter_context(tc.tile_pool(name="bands", bufs=4))

    comb = const.tile([128, 16, C], bf16)        # 16 combined taps, [i, slot, o]
    demP = const.tile([128, 1], f32)             # rsqrt demod, per output chan
    dmb = const.tile([128, 2], f32)              # de_mod[i] | bias[o]

    # ---- input bands: host-padded rows [S, S+rows+2) land contiguously; one
    # descriptor per partition.  Bands 0/1 ride the sync HWDGE ring (boots
    # first), the rest the gpsimd SWDGE ring.
    band_tiles = []
    band_dmas = []
    for bb in range(NBANDS):
        n = BAND_ROWS[bb] + 2
        bt = bands.tile([128, n, WP], bf16, tag="band", name=f"band{bb}")
        band_tiles.append(bt)
        band_dmas.append((bt[:, 0:n, :], x[:, BAND_START[bb] : BAND_START[bb] + n, :]))

    with tc.tile_pool(name="wtmp", bufs=1) as wtmp, tc.tile_pool(
        name="wpsum", bufs=1, space="PSUM"
    ) as wpsum:
        LUTn = wtmp.tile([RANK, C], bf16)        # 0.25 * lora_up^T: [r, o]
        LD9 = wtmp.tile([RANK, 9, C], bf16)      # lora_down^T: [r, t, i]
        WbTS = wtmp.tile([128, 9, C], f32)       # Wb^T: [i, t, o]
        WbO = wtmp.tile([128, 9, C], f32)        # Wb: [o, t, i]
        dmO = wtmp.tile([128, 9, C], f32)        # de_mod[i] bcast over [o, t]

        # sync ring: smallest/most-urgent first
        nc.sync.dma_start(LUTn[:], luT[:])
        nc.sync.dma_start(LD9[:], ldT[:])
        nc.sync.dma_start(*band_dmas[0])
        nc.sync.dma_start(WbTS[:], wbT[:])
        nc.sync.dma_start(*band_dmas[1])
        # demod-path tensors (sync ring too; the ACT HWDGE ring exists in the
        # BIR model but hangs on this runtime)
        nc.sync.dma_start(dmb[:], dmb_d[:])
        nc.sync.dma_start(WbO[:], wbO[:])
        nc.sync.dma_start(dmO[:], dmO_d[:])
        # gpsimd SWDGE: band 2 fires now; bands 3+ recycle band buffers, so
        # their WAR-gated dma_starts are emitted AFTER gpsimd's compute ops
        # (a dma_start parked on a WAR semaphore would otherwise block the
        # comb-slot builds the PE needs to make progress -> deadlock).
        nc.gpsimd.dma_start(*band_dmas[2])

        # ---- PE early: lora deltas in both layouts, then prewarm.
        # deltaP[i, t, o] = sum_r down[r,i,t] * 0.25*up[o,r]
        deltaP = wpsum.tile([128, 9, C], f32)
        for t in range(9):
            nc.tensor.matmul(
                deltaP[:, t, :], LD9[:, t, :], LUTn[:], start=True, stop=True
            )
        # deltaO[o, (t,i)] in 3 chunks of 384 (PSUM bank = 512 f32)
        ld9f = LD9.rearrange("p t c -> p (t c)")
        deltaO = []
        if not _NODEMO:
            for k in range(3):
                dk = wpsum.tile([128, 384], f32, name=f"deltaO{k}")
                nc.tensor.matmul(
                    dk[:], LUTn[:], ld9f[:, 384 * k : 384 * (k + 1)],
                    start=True, stop=True,
                )
                deltaO.append(dk)
        # keep the PE continuously busy so the pstate ramp (3us to 2.4 GHz)
        # completes during the weight stage instead of the conv stream
        if not _NOWARM:
            warmP = wpsum.tile([128, 384], f32)
            for _ in range(N_WARM):
                nc.tensor.matmul(
                    warmP[:], LUTn[:], ld9f[:, 0:384], start=True, stop=True
                )

        # ---- ACT: WbTm[i,t,o] = Wb^T * de_mod[i]
        WbTm = wtmp.tile([128, 9, C], f32)
        if _NOACTMUL:
            nc.vector.tensor_scalar_mul(WbTm[:], WbTS[:], dmb[:, 0:1])
        else:
            nc.scalar.mul(WbTm[:], WbTS[:], dmb[:, 0:1])

        # ---- modulated weights wm3[i,t,o] = WbTm + deltaP*de_mod[i].
        # gpsimd cannot read PSUM, so every PSUM-consuming op lives on DVE;
        # gpsimd gets the SBUF-only combine work.
        gps = nc.vector if _NOGPS else nc.gpsimd
        wm3 = wtmp.tile([128, 9, C], f32)
        nc.vector.scalar_tensor_tensor(
            wm3[:], deltaP[:], dmb[:, 0:1], WbTm[:],
            op0=ALU.mult, op1=ALU.add,
        )

        # 16 combined tap matrices.  Row combos over ki (t = 3*ki + kj):
        #   (di=0, a=0): ki0        (di=0, a=1): ki1+ki2
        #   (di=1, a=0): ki0+ki1    (di=1, a=1): ki2
        # and the same pattern over kj for (dj, b).
        R01 = wtmp.tile([128, 3, C], f32)
        nc.vector.tensor_add(R01[:], wm3[:, 3:6, :], wm3[:, 6:9, :])
        R10 = wtmp.tile([128, 3, C], f32)
        gps.tensor_add(R10[:], wm3[:, 0:3, :], wm3[:, 3:6, :])
        rowsrc = {
            (0, 0): wm3[:, 0:3, :],
            (0, 1): R01[:],
            (1, 0): R10[:],
            (1, 1): wm3[:, 6:9, :],
        }

        # comb slot layout: slot = 8*di + 2*a + 4*dj + b (bf16 on write).
        # DVE builds phases 0/2, gpsimd phases 1/3, in PE-use order.
        def build_slot(eng, p, q):
            di, dj = p >> 1, p & 1
            a, b = q >> 1, q & 1
            S = rowsrc[(di, a)]
            dst = comb[:, 8 * di + 2 * a + 4 * dj + b, :]
            if dj == 0 and b == 0:
                eng.tensor_copy(dst, S[:, 0, :])
            elif dj == 1 and b == 1:
                eng.tensor_copy(dst, S[:, 2, :])
            elif dj == 0:
                eng.tensor_add(dst, S[:, 1, :], S[:, 2, :])
            else:
                eng.tensor_add(dst, S[:, 0, :], S[:, 1, :])

        # ---- demod in the [o, (t,i)] layout: free-axis reduce, no PE.
        # DVE drains the deltaO PSUM banks; gpsimd applies the modulation.
        wO = wtmp.tile([128, 9, C], f32)
        wOf = wO.rearrange("p t c -> p (t c)")
        WbOf = WbO.rearrange("p t c -> p (t c)")

        for q in range(4):
            build_slot(nc.vector, 0, q)
        if not _NODEMO:
            for k in range(3):
                nc.vector.tensor_add(
                    wOf[:, 384 * k : 384 * (k + 1)],
                    deltaO[k][:],
                    WbOf[:, 384 * k : 384 * (k + 1)],
                )
        for q in range(4):
            build_slot(gps, 1, q)
        for q in range(4):
            build_slot(gps, 3, q)
        wmO = wtmp.tile([128, 9, C], f32)
        if not _NODEMO:
            gps.tensor_tensor(wmO[:], wO[:], dmO[:], op=ALU.mult)
        for q in range(4):
            build_slot(nc.vector, 2, q)
        if _NODEMO:
            nc.vector.memset(demP[:], 1.0)
        else:
            sqscr = wtmp.tile([128, 9, C], f32)
            nc.scalar.square(sqscr[:], wmO[:])
            s2o = wtmp.tile([128, 1], f32)
            nc.vector.tensor_reduce(
                s2o[:], sqscr.rearrange("p t c -> p (t c)"), axis=AX.X, op=ALU.add
            )
            t1 = wtmp.tile([128, 1], f32)
            nc.vector.tensor_scalar_add(t1[:], s2o[:], EPS)
            t2 = wtmp.tile([128, 1], f32)
            nc.scalar.sqrt(t2[:], t1[:])
            nc.vector.reciprocal(demP[:], t2[:])

        # now it is safe to park gpsimd on the recycled-band WAR semaphores
        for bb in range(3, NBANDS):
            nc.gpsimd.dma_start(*band_dmas[bb])

    # ---- main conv loop ----
    mpsum = ctx.enter_context(tc.tile_pool(name="mpsum", bufs=8, space="PSUM"))
    opool = ctx.enter_context(tc.tile_pool(name="obuf", bufs=3))

    # blocks of 3 x-rows; pairs of blocks share one output staging tile so
    # the sync sequencer issues half as many (128-descriptor) output DMAs.
    blk_band = []
    for g in range(NBLOCKS):
        i0 = g * R_BLK
        bb = next(
            b for b in range(NBANDS)
            if BAND_START[b] <= i0 and i0 + min(R_BLK, H - i0) <= BAND_START[b] + BAND_ROWS[b]
        )
        blk_band.append(bb)

    ob = None
    for g in range(NBLOCKS):
        i0 = g * R_BLK
        R = min(R_BLK, H - i0)
        bb = blk_band[g]
        bt = band_tiles[bb]
        ph = []
        for p in range(4):
            di, dj = p >> 1, p & 1
            pt = mpsum.tile([128, R * W], f32, tag="ph", name=f"ph{p}_{i0}")
            for q in range(4):
                a, b = q >> 1, q & 1
                tr = i0 + a + di - BAND_START[bb]     # padded-tile row
                co = b + dj                           # padded-tile col
                rhs = bt[:, tr : tr + R, co : co + W]
                slot = 8 * di + 2 * a + 4 * dj + b
                nc.tensor.matmul(
                    pt[:], comb[:, slot, :], rhs,
                    start=(q == 0), stop=(q == 3),
                )
            ph.append(pt)

        if g % 2 == 0:
            RR = R + (min(R_BLK, H - i0 - R) if g + 1 < NBLOCKS else 0)
            ob = opool.tile(
                [128, RR, 2, 2 * W], f32 if _F32OUT else bf16, tag="ob", name=f"ob_{i0}"
            )
            lr0 = 0
        else:
            lr0 = R_BLK
        obv = ob.rearrange("p r d (j two) -> p r d two j", two=2)
        # interleave phases into full output rows; scale by demod, add bias
        for p in range(4):
            di, dj = p >> 1, p & 1
            dst = obv[:, lr0 : lr0 + R, di, dj, :]
            srcv = ph[p].rearrange("p (r j) -> p r j", r=R)
            if dj == 0:
                nc.vector.tensor_scalar(
                    dst, srcv, demP[:, 0:1], dmb[:, 1:2],
                    op0=ALU.mult, op1=ALU.add,
                )
            else:
                nc.scalar.activation(
                    dst, srcv, AF.Identity, bias=dmb[:, 1:2], scale=demP[:, 0:1]
                )
        if g % 2 == 1 or g == NBLOCKS - 1:
            g0 = g - (g % 2)
            y0 = 2 * g0 * R_BLK
            nrows = 2 * ob.shape[1]
            nc.sync.dma_start(y[:, y0 : y0 + nrows, :], ob[:])


def _build():
    nc = bacc.Bacc(
        "TRN2",
        target_bir_lowering=False,
        debug=False,
        enable_asserts=False,
        num_devices=NCORES,
    )
    x = nc.dram_tensor("x", [C, HP, WP], bf16, kind="ExternalInput").ap()
    dmb_d = nc.dram_tensor("dmb", [C, 2], f32, kind="ExternalInput").ap()
    dmO_d = nc.dram_tensor("dmO", [C, 9 * C], f32, kind="ExternalInput").ap()
    wbT = nc.dram_tensor("WbT", [C, 9 * C], f32, kind="ExternalInput").ap()
    wbO = nc.dram_tensor("WbO", [C, 9 * C], f32, kind="ExternalInput").ap()
    luT = nc.dram_tensor("lora_upT", [RANK, C], bf16, kind="ExternalInput").ap()
    ldT = nc.dram_tensor("lora_downT", [RANK, 9 * C], bf16, kind="ExternalInput").ap()
    y = nc.dram_tensor(
        "y", [C, 2 * H, 2 * W], f32 if _F32OUT else bf16, kind="ExternalOutput"
    ).ap()

    with tile.TileContext(nc) as tc:
        with ExitStack() as ctx:
            _conv_kernel(ctx, tc, y, x, dmb_d, dmO_d, wbT, wbO, luT, ldT)
    nc.compile()
    return nc


_CACHE = {}


def _get_nc():
    if "nc" not in _CACHE:
        _CACHE["nc"] = _build()
    return _CACHE["nc"]


def _make_in_maps(x, de_mod, Wb, lora_up, lora_down, bias):
    x = np.asarray(x, dtype=np.float32)
    de_mod = np.asarray(de_mod, dtype=np.float32)
    Wb = np.asarray(Wb, dtype=np.float32)
    lora_up = np.asarray(lora_up, dtype=np.float32)
    lora_down = np.asarray(lora_down, dtype=np.float32)
    bias = np.asarray(bias, dtype=np.float32).reshape(C)
    # layout-only host prep: pad + cast x, transpose/replicate weights
    xp = np.zeros((B, C, HP, WP), dtype=ml_dtypes.bfloat16)
    xp[:, :, 1 : 1 + H, 1 : 1 + W] = x.astype(ml_dtypes.bfloat16)
    wbT = np.ascontiguousarray(Wb.transpose(1, 2, 3, 0).reshape(C, 9 * C))
    wbO = np.ascontiguousarray(Wb.transpose(0, 2, 3, 1).reshape(C, 9 * C))
    luT = np.ascontiguousarray((SCALING * lora_up).T).astype(ml_dtypes.bfloat16)
    ldT = np.ascontiguousarray(
        lora_down.transpose(0, 2, 3, 1).reshape(RANK, 9 * C)
    ).astype(ml_dtypes.bfloat16)
    in_maps = []
    for b in range(NCORES):
        dmO = np.ascontiguousarray(
            np.broadcast_to(np.tile(de_mod[b], 9)[None, :], (C, 9 * C))
        )
        in_maps.append(
            {
                "x": np.ascontiguousarray(xp[b]),
                "dmb": np.ascontiguousarray(np.stack([de_mod[b], bias], axis=1)),
                "dmO": dmO,
                "WbT": wbT,
                "WbO": wbO,
                "lora_upT": luT,
                "lora_downT": ldT,
            }
        )
    return in_maps


def run(inputs, trace=False, trace_kwargs=None):
    nc = _get_nc()
    in_maps = _make_in_maps(**inputs)
    res = run_bass_kernel_spmd(
        nc,
        in_maps,
        core_ids=list(range(NCORES)),
        trace=trace,
        **(trace_kwargs or {}),
    )
    y = np.stack(
        [res.results[b]["y"].astype(np.float32) for b in range(NCORES)], axis=0
    )
    return y, res


def kernel(**inputs):
    y, _ = run(inputs)
    return y


# revision 13
# speedup vs baseline: 1.0651x; 1.0120x over previous
"""Trainium2 Bass kernel for nn_NeuronS3DiffUpsample2D.

Reference computation (per sample b):
    up   = nearest-2x-upsample(x[b])                       # [C, 320, 320]
    w    = Wb + 0.25 * einsum('or,rikl->oikl', lora_up, lora_down)
    w_b  = w * de_mod[b, None, :, None, None]              # modulate input chans
    dem  = rsqrt(sum_{i,k,l} w_b^2 + eps)                  # per output chan
    y[b] = conv2d(up, w_b, SAME) * dem + bias

Key algebraic transform: a 3x3 SAME conv on a 2x nearest-upsampled image
decomposes into 4 output phases (di, dj in {0,1}), each a 2x2 conv on the
ORIGINAL 160x160 input:
    y[2i+di, 2j+dj] = sum_{a,b in {0,1}} K[di,dj,a,b] @ x[i+a+di-1, j+b+dj-1]
where the 16 [O, I] matrices K are sums of 1/2/4 of the 9 taps of w.
This is 4/9 of the naive FLOPs and never materializes the upsampled image.
The demod scale is per output channel and conv is linear in w, so the conv
OUTPUT is scaled by dem[o] at PSUM eviction, fused with the bias add.

All tensors ride bf16 (rel-err budget is 2e-2; bf16 end-to-end lands ~4e-3):
x is host-padded to [C,162,162] bf16 so every band DMA is one contiguous
descriptor per partition (no per-row 640B descriptors, no on-device border
zeroing), matmuls are bf16 (1 cycle/row at any free size - also kills the
f32r N<256 4x penalty on the R=1 tail block), and y is written bf16 and
upcast on the host.  Total HBM traffic drops from 66 MB to ~35 MB per core,
taking DMA well below the PE's ~173 us of matmul streaming.

The demod reduction runs in a second [o, (t,i)] weight layout so the
partition-axis sum becomes a free-axis reduce: 3 small early matmuls build
delta in [o,.] form, then gpsimd/DVE/ACT finish rsqrt off the PE's program
order (the baseline's ones-matmul made the PE wait ~10us on a DVE chain).
Dummy "warm" matmuls keep the PE busy through the weight stage so the
2.4 GHz pstate ramp completes before the main conv stream starts.

Sharding: data-parallel over batch B=8 across 8 NeuronCores; each core
builds its own per-sample weights locally.  Host-side work is layout only:
padding, per-sample slicing/replication, transposes, dtype casts.
"""

import os
import sys
import numpy as np
from contextlib import ExitStack

_NOWARM = bool(int(os.environ.get("K_NOWARM", "0")))
_NOGPS = bool(int(os.environ.get("K_NOGPS", "0")))
_NODEMO = bool(int(os.environ.get("K_NODEMO", "0")))   # demod=1.0 (wrong numerics, hang-test only)
_F32OUT = bool(int(os.environ.get("K_F32OUT", "0")))
_NOACTMUL = bool(int(os.environ.get("K_NOACTMUL", "0")))

try:
    import concourse.bass as bass
except ImportError:  # grading env without the axon PYTHONPATH
    sys.path.insert(0, "/opt/trn_rl_repo")
    import concourse.bass as bass
import ml_dtypes
import concourse.tile as tile
from concourse import bacc, mybir
from concourse.bass_utils import run_bass_kernel_spmd

B, C, H, W = 8, 128, 160, 160
RANK = 32
SCALING = 0.25
EPS = 1e-8
HP, WP = H + 2, W + 2      # host-padded input (zero border baked in)
R_BLK = 3                  # x-rows per matmul block -> N = 3*160 = 480 <= 512
# x-row band split: band 0 tiny so the first matmuls gate only on ~0.3 MB.
BAND_ROWS = [6, 27, 27, 27, 27, 27, 19]
BAND_START = [0, 6, 33, 60, 87, 114, 141]
NBANDS = len(BAND_ROWS)
NBLOCKS = 54               # 53 blocks of 3 rows + 1 tail row
N_WARM = 14                # PE pstate prewarm matmuls
NCORES = 8

f32 = mybir.dt.float32
bf16 = mybir.dt.bfloat16


def _conv_kernel(ctx, tc, y, x, dmb_d, wbT, ident_d, luT, ldT):
    nc = tc.nc
    AF = mybir.ActivationFunctionType
    ALU = mybir.AluOpType
    AX = mybir.AxisListType

    const = ctx.enter_context(tc.tile_pool(name="const", bufs=1))
    bands = ctx.enter_context(tc.tile_pool(name="bands", bufs=4))

    comb = const.tile([128, 16, C], bf16)        # 16 combined taps, [i, slot, o]
    demP = const.tile([128, 1], f32)             # rsqrt demod, per output chan
    dmb = const.tile([128, 2], f32)              # de_mod[i] | bias[o]

    # ---- input bands: host-padded rows [S, S+rows+2) land contiguously; one
    # descriptor per partition.  Bands 0/1 ride the sync HWDGE ring (boots
    # first), the rest the gpsimd SWDGE ring.
    band_tiles = []
    band_dmas = []
    for bb in range(NBANDS):
        n = BAND_ROWS[bb] + 2
        bt = bands.tile([128, n, WP], bf16, tag="band", name=f"band{bb}")
        band_tiles.append(bt)
        band_dmas.append((bt[:, 0:n, :], x[:, BAND_START[bb] : BAND_START[bb] + n, :]))

    with tc.tile_pool(name="wtmp", bufs=1) as wtmp, tc.tile_pool(
        name="wpsum", bufs=1, space="PSUM"
    ) as wpsum:
        LUTn = wtmp.tile([RANK, C], bf16)        # 0.25 * lora_up^T: [r, o]
        LD9 = wtmp.tile([RANK, 9, C], bf16)      # lora_down^T: [r, t, i]
        WbTS = wtmp.tile([128, 9, C], f32)       # Wb^T: [i, t, o]
        ident = wtmp.tile([128, 128], f32)       # PE-transpose identity

        # sync ring: smallest/most-urgent first
        nc.sync.dma_start(LUTn[:], luT[:])
        nc.sync.dma_start(LD9[:], ldT[:])
        nc.sync.dma_start(*band_dmas[0])
        nc.sync.dma_start(WbTS[:], wbT[:])
        nc.sync.dma_start(*band_dmas[1])
        # demod-path tensors (sync ring too; the ACT HWDGE ring exists in the
        # BIR model but hangs on this runtime)
        nc.sync.dma_start(dmb[:], dmb_d[:])
        nc.sync.dma_start(ident[:], ident_d[:])
        # gpsimd SWDGE: band 2 fires now; bands 3+ recycle band buffers, so
        # their WAR-gated dma_starts are emitted AFTER gpsimd's compute ops
        # (a dma_start parked on a WAR semaphore would otherwise block the
        # comb-slot builds the PE needs to make progress -> deadlock).
        nc.gpsimd.dma_start(*band_dmas[2])

        # ---- PE early: lora deltas in both layouts, then prewarm.
        # deltaP[i, t, o] = sum_r down[r,i,t] * 0.25*up[o,r]
        deltaP = wpsum.tile([128, 9, C], f32)
        for t in range(9):
            nc.tensor.matmul(
                deltaP[:, t, :], LD9[:, t, :], LUTn[:], start=True, stop=True
            )
        ld9f = LD9.rearrange("p t c -> p (t c)")
        # keep the PE continuously busy so the pstate ramp (3us to 2.4 GHz)
        # completes during the weight stage instead of the conv stream
        if not _NOWARM:
            warmP = wpsum.tile([128, 384], f32)
            for _ in range(N_WARM):
                nc.tensor.matmul(
                    warmP[:], LUTn[:], ld9f[:, 0:384], start=True, stop=True
                )

        # ---- ACT: WbTm[i,t,o] = Wb^T * de_mod[i]
        WbTm = wtmp.tile([128, 9, C], f32)
        if _NOACTMUL:
            nc.vector.tensor_scalar_mul(WbTm[:], WbTS[:], dmb[:, 0:1])
        else:
            nc.scalar.mul(WbTm[:], WbTS[:], dmb[:, 0:1])

        # ---- modulated weights wm3[i,t,o] = WbTm + deltaP*de_mod[i].
        # gpsimd cannot read PSUM, so every PSUM-consuming op lives on DVE;
        # gpsimd gets the SBUF-only combine work.
        gps = nc.vector if _NOGPS else nc.gpsimd
        wm3 = wtmp.tile([128, 9, C], f32)
        nc.vector.scalar_tensor_tensor(
            wm3[:], deltaP[:], dmb[:, 0:1], WbTm[:],
            op0=ALU.mult, op1=ALU.add,
        )

        # 16 combined tap matrices.  Row combos over ki (t = 3*ki + kj):
        #   (di=0, a=0): ki0        (di=0, a=1): ki1+ki2
        #   (di=1, a=0): ki0+ki1    (di=1, a=1): ki2
        # and the same pattern over kj for (dj, b).
        R01 = wtmp.tile([128, 3, C], f32)
        nc.vector.tensor_add(R01[:], wm3[:, 3:6, :], wm3[:, 6:9, :])
        R10 = wtmp.tile([128, 3, C], f32)
        gps.tensor_add(R10[:], wm3[:, 0:3, :], wm3[:, 3:6, :])
        rowsrc = {
            (0, 0): wm3[:, 0:3, :],
            (0, 1): R01[:],
            (1, 0): R10[:],
            (1, 1): wm3[:, 6:9, :],
        }

        # comb slot layout: slot = 8*di + 2*a + 4*dj + b (bf16 on write).
        # DVE builds phases 0/2, gpsimd phases 1/3, in PE-use order.
        def build_slot(eng, p, q):
            di, dj = p >> 1, p & 1
            a, b = q >> 1, q & 1
            S = rowsrc[(di, a)]
            dst = comb[:, 8 * di + 2 * a + 4 * dj + b, :]
            if dj == 0 and b == 0:
                eng.tensor_copy(dst, S[:, 0, :])
            elif dj == 1 and b == 1:
                eng.tensor_copy(dst, S[:, 2, :])
            elif dj == 0:
                eng.tensor_add(dst, S[:, 1, :], S[:, 2, :])
            else:
                eng.tensor_add(dst, S[:, 0, :], S[:, 1, :])

        for q in range(4):
            build_slot(nc.vector, 0, q)
        for q in range(4):
            build_slot(gps, 1, q)
        for q in range(4):
            build_slot(gps, 3, q)
        for q in range(4):
            build_slot(nc.vector, 2, q)

        # ---- demod: PE-transpose wm3 into [o, (t,i)] PSUM, then one ACT
        # Square pass whose accum_out IS the free-axis sum.  No extra DMAs,
        # no partition reduction, and the only PE cost is ~1us of transposes
        # emitted after the prewarm (wm3 is ready by then).
        if not _NODEMO:
            wmOT = wpsum.tile([128, 9, C], f32)
            for t in range(9):
                nc.tensor.transpose(wmOT[:, t, :], wm3[:, t, :], ident[:])
        if _NODEMO:
            nc.vector.memset(demP[:], 1.0)
        else:
            sqscr = wtmp.tile([128, 9, C], f32)
            s2o = wtmp.tile([128, 1], f32)
            nc.scalar.activation(
                sqscr[:], wmOT[:], AF.Square, accum_out=s2o[:]
            )
            t1 = wtmp.tile([128, 1], f32)
            nc.vector.tensor_scalar_add(t1[:], s2o[:], EPS)
            t2 = wtmp.tile([128, 1], f32)
            nc.scalar.sqrt(t2[:], t1[:])
            nc.vector.reciprocal(demP[:], t2[:])

        # now it is safe to park gpsimd on the recycled-band WAR semaphores
        for bb in range(3, NBANDS):
            nc.gpsimd.dma_start(*band_dmas[bb])

    # ---- main conv loop ----
    mpsum = ctx.enter_context(tc.tile_pool(name="mpsum", bufs=8, space="PSUM"))
    opool = ctx.enter_context(tc.tile_pool(name="obuf", bufs=3))

    # blocks of 3 x-rows; pairs of blocks share one output staging tile so
    # the sync sequencer issues half as many (128-descriptor) output DMAs.
    blk_band = []
    for g in range(NBLOCKS):
        i0 = g * R_BLK
        bb = next(
            b for b in range(NBANDS)
            if BAND_START[b] <= i0 and i0 + min(R_BLK, H - i0) <= BAND_START[b] + BAND_ROWS[b]
        )
        blk_band.append(bb)

    ob = None
    for g in range(NBLOCKS):
        i0 = g * R_BLK
        R = min(R_BLK, H - i0)
        bb = blk_band[g]
        bt = band_tiles[bb]
        ph = []
        for p in range(4):
            di, dj = p >> 1, p & 1
            pt = mpsum.tile([128, R * W], f32, tag="ph", name=f"ph{p}_{i0}")
            for q in range(4):
                a, b = q >> 1, q & 1
                tr = i0 + a + di - BAND_START[bb]     # padded-tile row
                co = b + dj                           # padded-tile col
                rhs = bt[:, tr : tr + R, co : co + W]
                slot = 8 * di + 2 * a + 4 * dj + b
                nc.tensor.matmul(
                    pt[:], comb[:, slot, :], rhs,
                    start=(q == 0), stop=(q == 3),
                )
            ph.append(pt)

        if g % 2 == 0:
            RR = R + (min(R_BLK, H - i0 - R) if g + 1 < NBLOCKS else 0)
            ob = opool.tile(
                [128, RR, 2, 2 * W], f32 if _F32OUT else bf16, tag="ob", name=f"ob_{i0}"
            )
            lr0 = 0
        else:
            lr0 = R_BLK
        obv = ob.rearrange("p r d (j two) -> p r d two j", two=2)
        # interleave phases into full output rows; scale by demod, add bias
        for p in range(4):
            di, dj = p >> 1, p & 1
            dst = obv[:, lr0 : lr0 + R, di, dj, :]
            srcv = ph[p].rearrange("p (r j) -> p r j", r=R)
            if dj == 0:
                nc.vector.tensor_scalar(
                    dst, srcv, demP[:, 0:1], dmb[:, 1:2],
                    op0=ALU.mult, op1=ALU.add,
                )
            else:
                nc.scalar.activation(
                    dst, srcv, AF.Identity, bias=dmb[:, 1:2], scale=demP[:, 0:1]
                )
        if g % 2 == 1 or g == NBLOCKS - 1:
            g0 = g - (g % 2)
            y0 = 2 * g0 * R_BLK
            nrows = 2 * ob.shape[1]
            nc.sync.dma_start(y[:, y0 : y0 + nrows, :], ob[:])


def _build():
    nc = bacc.Bacc(
        "TRN2",
        target_bir_lowering=False,
        debug=False,
        enable_asserts=False,
        num_devices=NCORES,
    )
    x = nc.dram_tensor("x", [C, HP, WP], bf16, kind="ExternalInput").ap()
    dmb_d = nc.dram_tensor("dmb", [C, 2], f32, kind="ExternalInput").ap()
    wbT = nc.dram_tensor("WbT", [C, 9 * C], f32, kind="ExternalInput").ap()
    ident_d = nc.dram_tensor("ident", [128, 128], f32, kind="ExternalInput").ap()
    luT = nc.dram_tensor("lora_upT", [RANK, C], bf16, kind="ExternalInput").ap()
    ldT = nc.dram_tensor("lora_downT", [RANK, 9 * C], bf16, kind="ExternalInput").ap()
    y = nc.dram_tensor(
        "y", [C, 2 * H, 2 * W], f32 if _F32OUT else bf16, kind="ExternalOutput"
    ).ap()

    with tile.TileContext(nc) as tc:
        with ExitStack() as ctx:
            _conv_kernel(ctx, tc, y, x, dmb_d, wbT, ident_d, luT, ldT)
    nc.compile()
    return nc


_CACHE = {}


def _get_nc():
    if "nc" not in _CACHE:
        _CACHE["nc"] = _build()
    return _CACHE["nc"]


def _make_in_maps(x, de_mod, Wb, lora_up, lora_down, bias):
    x = np.asarray(x, dtype=np.float32)
    de_mod = np.asarray(de_mod, dtype=np.float32)
    Wb = np.asarray(Wb, dtype=np.float32)
    lora_up = np.asarray(lora_up, dtype=np.float32)
    lora_down = np.asarray(lora_down, dtype=np.float32)
    bias = np.asarray(bias, dtype=np.float32).reshape(C)
    # layout-only host prep: pad + cast x, transpose/replicate weights
    xp = np.zeros((B, C, HP, WP), dtype=ml_dtypes.bfloat16)
    xp[:, :, 1 : 1 + H, 1 : 1 + W] = x.astype(ml_dtypes.bfloat16)
    wbT = np.ascontiguousarray(Wb.transpose(1, 2, 3, 0).reshape(C, 9 * C))
    luT = np.ascontiguousarray((SCALING * lora_up).T).astype(ml_dtypes.bfloat16)
    ldT = np.ascontiguousarray(
        lora_down.transpose(0, 2, 3, 1).reshape(RANK, 9 * C)
    ).astype(ml_dtypes.bfloat16)
    ident = np.eye(128, dtype=np.float32)
    in_maps = []
    for b in range(NCORES):
        in_maps.append(
            {
                "x": np.ascontiguousarray(xp[b]),
                "dmb": np.ascontiguousarray(np.stack([de_mod[b], bias], axis=1)),
                "WbT": wbT,
                "ident": ident,
                "lora_upT": luT,
                "lora_downT": ldT,
            }
        )
    return in_maps


def run(inputs, trace=False, trace_kwargs=None):
    nc = _get_nc()
    in_maps = _make_in_maps(**inputs)
    res = run_bass_kernel_spmd(
        nc,
        in_maps,
        core_ids=list(range(NCORES)),
        trace=trace,
        **(trace_kwargs or {}),
    )
    y = np.stack(
        [res.results[b]["y"].astype(np.float32) for b in range(NCORES)], axis=0
    )
    return y, res


def kernel(**inputs):
    y, _ = run(inputs)
    return y


# revision 17
# speedup vs baseline: 1.0684x; 1.0031x over previous
"""Trainium2 Bass kernel for nn_NeuronS3DiffUpsample2D.

Reference computation (per sample b):
    up   = nearest-2x-upsample(x[b])                       # [C, 320, 320]
    w    = Wb + 0.25 * einsum('or,rikl->oikl', lora_up, lora_down)
    w_b  = w * de_mod[b, None, :, None, None]              # modulate input chans
    dem  = rsqrt(sum_{i,k,l} w_b^2 + eps)                  # per output chan
    y[b] = conv2d(up, w_b, SAME) * dem + bias

Key algebraic transform: a 3x3 SAME conv on a 2x nearest-upsampled image
decomposes into 4 output phases (di, dj in {0,1}), each a 2x2 conv on the
ORIGINAL 160x160 input:
    y[2i+di, 2j+dj] = sum_{a,b in {0,1}} K[di,dj,a,b] @ x[i+a+di-1, j+b+dj-1]
where the 16 [O, I] matrices K are sums of 1/2/4 of the 9 taps of w.
This is 4/9 of the naive FLOPs and never materializes the upsampled image.
The demod scale is per output channel and conv is linear in w, so the conv
OUTPUT is scaled by dem[o] at PSUM eviction, fused with the bias add.

All tensors ride bf16 (rel-err budget is 2e-2; bf16 end-to-end lands ~4e-3):
x is host-padded to [C,162,162] bf16 so every band DMA is one contiguous
descriptor per partition (no per-row 640B descriptors, no on-device border
zeroing), matmuls are bf16 (1 cycle/row at any free size - also kills the
f32r N<256 4x penalty on the R=1 tail block), and y is written bf16 and
upcast on the host.  Total HBM traffic drops from 66 MB to ~35 MB per core,
taking DMA well below the PE's ~173 us of matmul streaming.

The demod reduction runs in a second [o, (t,i)] weight layout so the
partition-axis sum becomes a free-axis reduce: 3 small early matmuls build
delta in [o,.] form, then gpsimd/DVE/ACT finish rsqrt off the PE's program
order (the baseline's ones-matmul made the PE wait ~10us on a DVE chain).
Dummy "warm" matmuls keep the PE busy through the weight stage so the
2.4 GHz pstate ramp completes before the main conv stream starts.

Sharding: data-parallel over batch B=8 across 8 NeuronCores; each core
builds its own per-sample weights locally.  Host-side work is layout only:
padding, per-sample slicing/replication, transposes, dtype casts.
"""

import os
import sys
import numpy as np
from contextlib import ExitStack

_NOWARM = bool(int(os.environ.get("K_NOWARM", "0")))
_NOGPS = bool(int(os.environ.get("K_NOGPS", "0")))
_NODEMO = bool(int(os.environ.get("K_NODEMO", "0")))   # demod=1.0 (wrong numerics, hang-test only)
_F32OUT = bool(int(os.environ.get("K_F32OUT", "0")))
_NOACTMUL = bool(int(os.environ.get("K_NOACTMUL", "0")))
_DEBUG = bool(int(os.environ.get("K_DEBUG", "0")))

try:
    import concourse.bass as bass
except ImportError:  # grading env without the axon PYTHONPATH
    sys.path.insert(0, "/opt/trn_rl_repo")
    import concourse.bass as bass
import ml_dtypes
import concourse.tile as tile
from concourse import bacc, mybir
from concourse.bass_utils import run_bass_kernel_spmd

B, C, H, W = 8, 128, 160, 160
RANK = 32
SCALING = 0.25
EPS = 1e-8
HP, WP = H + 2, W + 2      # host-padded input (zero border baked in)
R_BLK = 3                  # x-rows per matmul block -> N = 3*160 = 480 <= 512
# x-row band split: band 0 tiny so the first matmuls gate only on ~0.3 MB.
BAND_ROWS = [6, 27, 27, 27, 27, 27, 19]
BAND_START = [0, 6, 33, 60, 87, 114, 141]
NBANDS = len(BAND_ROWS)
NBLOCKS = 54               # 53 blocks of 3 rows + 1 tail row
N_WARM = 14                # PE pstate prewarm matmuls
NCORES = 8

f32 = mybir.dt.float32
bf16 = mybir.dt.bfloat16


def _conv_kernel(ctx, tc, y, x, wbT, ll_d, idm_d, dbg=None):
    nc = tc.nc
    AF = mybir.ActivationFunctionType
    ALU = mybir.AluOpType
    AX = mybir.AxisListType

    const = ctx.enter_context(tc.tile_pool(name="const", bufs=1))
    bands = ctx.enter_context(tc.tile_pool(name="bands", bufs=4))

    comb = const.tile([128, 16, C], bf16)        # 16 combined taps, [i, slot, o]
    demP = const.tile([128, 1], f32)             # rsqrt demod, per output chan
    biasT = const.tile([128, 1], f32)            # bias, copied out of IDm

    # ---- input bands: host-padded rows [S, S+rows+2) land contiguously; one
    # descriptor per partition.  Bands 0/1 ride the sync HWDGE ring (boots
    # first), the rest the gpsimd SWDGE ring.
    band_tiles = []
    band_dmas = []
    for bb in range(NBANDS):
        n = BAND_ROWS[bb] + 2
        bt = bands.tile([128, n, WP], bf16, tag="band", name=f"band{bb}")
        band_tiles.append(bt)
        band_dmas.append((bt[:, 0:n, :], x[:, BAND_START[bb] : BAND_START[bb] + n, :]))

    with tc.tile_pool(name="wtmp", bufs=1) as wtmp, tc.tile_pool(
        name="wpsum", bufs=1, space="PSUM"
    ) as wpsum:
        LL = wtmp.tile([RANK, 1280], bf16)       # lora_down^T (t,i) | 0.25*lora_up^T
        WbTS = wtmp.tile([128, 9, C], bf16)      # Wb^T: [i, t, o]
        IDm = wtmp.tile([128, 194], f32)         # f32 eye | de_mod,bias | bf16 eye
        LD9 = LL[:, 0:1152].rearrange("p (t c) -> p t c", c=C)
        LUTn = LL[:, 1152:1280]
        ident = IDm[:, 0:128]
        dmb = IDm[:, 128:130]
        identb = IDm[:, 130:194].bitcast(bf16)

        # sync ring: few DMA instructions, few descriptors, urgent first
        nc.sync.dma_start(LL[:], ll_d[:])
        nc.sync.dma_start(*band_dmas[0])
        nc.sync.dma_start(WbTS[:], wbT[:])
        nc.sync.dma_start(IDm[:], idm_d[:])
        nc.sync.dma_start(*band_dmas[1])
        # gpsimd SWDGE: band 2 fires now; bands 3+ recycle band buffers, so
        # their WAR-gated dma_starts are emitted AFTER gpsimd's compute ops
        # (a dma_start parked on a WAR semaphore would otherwise block the
        # comb-slot builds the PE needs to make progress -> deadlock).
        nc.gpsimd.dma_start(*band_dmas[2])

        # ---- PE early: prewarm first (only needs LL; keeps the PE busy so
        # the pstate ramp to 2.4 GHz completes during the weight stage), then
        # per tap an adjacent lora+identity pair accumulating
        #   deltaP[i,t,o] = sum_r down[r,i,t]*0.25*up[o,r] + Wb^T[i,t,o]
        # (folds the base weight in on the PE; wm3 is then a single de_mod
        # multiply on DVE).  The pairs must be adjacent: interleaving other
        # matmuls inside an open accumulation group drops the first write.
        ld9f = LL[:, 0:1152]
        if not _NOWARM:
            warmP = wpsum.tile([128, 384], f32)
            for _ in range(N_WARM):
                nc.tensor.matmul(
                    warmP[:], LUTn[:], ld9f[:, 0:384], start=True, stop=True
                )
        deltaP = wpsum.tile([128, 9, C], f32)
        for t in range(9):
            nc.tensor.matmul(
                deltaP[:, t, :], LD9[:, t, :], LUTn[:], start=True, stop=False
            )
            nc.tensor.matmul(
                deltaP[:, t, :], identb[:], WbTS[:, t, :], start=False, stop=True
            )

        # bias is read during evictions long after IDm's SBUF may recycle
        nc.scalar.copy(biasT[:], dmb[:, 1:2])

        # ---- modulated weights wm3[i,t,o] = (Wb^T + delta) * de_mod[i].
        # gpsimd cannot read PSUM, so every PSUM-consuming op lives on DVE;
        # gpsimd gets the SBUF-only combine work.
        gps = nc.vector if _NOGPS else nc.gpsimd
        wm3 = wtmp.tile([128, 9, C], f32)
        nc.vector.tensor_scalar_mul(wm3[:], deltaP[:], dmb[:, 0:1])

        # 16 combined tap matrices.  Row combos over ki (t = 3*ki + kj):
        #   (di=0, a=0): ki0        (di=0, a=1): ki1+ki2
        #   (di=1, a=0): ki0+ki1    (di=1, a=1): ki2
        # and the same pattern over kj for (dj, b).
        R01 = wtmp.tile([128, 3, C], f32)
        nc.vector.tensor_add(R01[:], wm3[:, 3:6, :], wm3[:, 6:9, :])
        R10 = wtmp.tile([128, 3, C], f32)
        gps.tensor_add(R10[:], wm3[:, 0:3, :], wm3[:, 3:6, :])
        rowsrc = {
            (0, 0): wm3[:, 0:3, :],
            (0, 1): R01[:],
            (1, 0): R10[:],
            (1, 1): wm3[:, 6:9, :],
        }

        # comb slot layout: slot = 8*di + 2*a + 4*dj + b (bf16 on write).
        # DVE builds phases 0/2, gpsimd phases 1/3, in PE-use order.
        def build_slot(eng, p, q):
            di, dj = p >> 1, p & 1
            a, b = q >> 1, q & 1
            S = rowsrc[(di, a)]
            dst = comb[:, 8 * di + 2 * a + 4 * dj + b, :]
            if dj == 0 and b == 0:
                eng.tensor_copy(dst, S[:, 0, :])
            elif dj == 1 and b == 1:
                eng.tensor_copy(dst, S[:, 2, :])
            elif dj == 0:
                eng.tensor_add(dst, S[:, 1, :], S[:, 2, :])
            else:
                eng.tensor_add(dst, S[:, 0, :], S[:, 1, :])

        for q in range(4):
            build_slot(nc.vector, 0, q)
        for q in range(4):
            build_slot(gps, 1, q)
        for q in range(4):
            build_slot(gps, 3, q)
        for q in range(4):
            build_slot(nc.vector, 2, q)

        # ---- demod: PE-transpose wm3 into [o, (t,i)] PSUM, then one ACT
        # Square pass whose accum_out IS the free-axis sum.  No extra DMAs,
        # no partition reduction, and the only PE cost is ~1us of transposes
        # emitted after the prewarm (wm3 is ready by then).
        if not _NODEMO:
            wmOT = wpsum.tile([128, 9, C], f32)
            for t in range(9):
                nc.tensor.transpose(wmOT[:, t, :], wm3[:, t, :], ident[:])
        if _NODEMO:
            nc.vector.memset(demP[:], 1.0)
        else:
            sqscr = wtmp.tile([128, 9, C], f32)
            s2o = wtmp.tile([128, 1], f32)
            nc.scalar.activation(
                sqscr[:], wmOT[:], AF.Square, accum_out=s2o[:]
            )
            t1 = wtmp.tile([128, 1], f32)
            nc.vector.tensor_scalar_add(t1[:], s2o[:], EPS)
            t2 = wtmp.tile([128, 1], f32)
            nc.scalar.sqrt(t2[:], t1[:])
            nc.vector.reciprocal(demP[:], t2[:])

        if dbg is not None:
            nc.sync.dma_start(dbg["wm3"][:], wm3[:])
            nc.sync.dma_start(dbg["comb"][:], comb.rearrange("p s c -> p (s c)"))
            nc.sync.dma_start(dbg["demP"][:], demP[:])

        # now it is safe to park gpsimd on the recycled-band WAR semaphores
        for bb in range(3, NBANDS):
            nc.gpsimd.dma_start(*band_dmas[bb])

    # ---- main conv loop ----
    mpsum = ctx.enter_context(tc.tile_pool(name="mpsum", bufs=8, space="PSUM"))
    opool = ctx.enter_context(tc.tile_pool(name="obuf", bufs=3))

    # blocks of 3 x-rows; pairs of blocks share one output staging tile so
    # the sync sequencer issues half as many (128-descriptor) output DMAs.
    blk_band = []
    for g in range(NBLOCKS):
        i0 = g * R_BLK
        bb = next(
            b for b in range(NBANDS)
            if BAND_START[b] <= i0 and i0 + min(R_BLK, H - i0) <= BAND_START[b] + BAND_ROWS[b]
        )
        blk_band.append(bb)

    ob = None
    for g in range(NBLOCKS):
        i0 = g * R_BLK
        R = min(R_BLK, H - i0)
        bb = blk_band[g]
        bt = band_tiles[bb]
        ph = []
        for p in range(4):
            di, dj = p >> 1, p & 1
            pt = mpsum.tile([128, R * W], f32, tag="ph", name=f"ph{p}_{i0}")
            for q in range(4):
                a, b = q >> 1, q & 1
                tr = i0 + a + di - BAND_START[bb]     # padded-tile row
                co = b + dj                           # padded-tile col
                rhs = bt[:, tr : tr + R, co : co + W]
                slot = 8 * di + 2 * a + 4 * dj + b
                nc.tensor.matmul(
                    pt[:], comb[:, slot, :], rhs,
                    start=(q == 0), stop=(q == 3),
                )
            ph.append(pt)

        if g % 2 == 0:
            RR = R + (min(R_BLK, H - i0 - R) if g + 1 < NBLOCKS else 0)
            ob = opool.tile(
                [128, RR, 2, 2 * W], f32 if _F32OUT else bf16, tag="ob", name=f"ob_{i0}"
            )
            lr0 = 0
        else:
            lr0 = R_BLK
        obv = ob.rearrange("p r d (j two) -> p r d two j", two=2)
        # interleave phases into full output rows; scale by demod, add bias
        for p in range(4):
            di, dj = p >> 1, p & 1
            dst = obv[:, lr0 : lr0 + R, di, dj, :]
            srcv = ph[p].rearrange("p (r j) -> p r j", r=R)
            if dj == 0:
                nc.vector.tensor_scalar(
                    dst, srcv, demP[:, 0:1], biasT[:, 0:1],
                    op0=ALU.mult, op1=ALU.add,
                )
            else:
                nc.scalar.activation(
                    dst, srcv, AF.Identity, bias=biasT[:, 0:1], scale=demP[:, 0:1]
                )
        if g % 2 == 1 or g == NBLOCKS - 1:
            g0 = g - (g % 2)
            y0 = 2 * g0 * R_BLK
            nrows = 2 * ob.shape[1]
            nc.sync.dma_start(y[:, y0 : y0 + nrows, :], ob[:])


def _build():
    nc = bacc.Bacc(
        "TRN2",
        target_bir_lowering=False,
        debug=False,
        enable_asserts=False,
        num_devices=NCORES,
    )
    x = nc.dram_tensor("x", [C, HP, WP], bf16, kind="ExternalInput").ap()
    wbT = nc.dram_tensor("WbT", [C, 9 * C], bf16, kind="ExternalInput").ap()
    ll_d = nc.dram_tensor("LL", [RANK, 1280], bf16, kind="ExternalInput").ap()
    idm_d = nc.dram_tensor("IDm", [128, 194], f32, kind="ExternalInput").ap()
    y = nc.dram_tensor(
        "y", [C, 2 * H, 2 * W], f32 if _F32OUT else bf16, kind="ExternalOutput"
    ).ap()
    dbg = None
    if _DEBUG:
        dbg = {
            "wm3": nc.dram_tensor("dbg_wm3", [C, 9 * C], f32, kind="ExternalOutput").ap(),
            "comb": nc.dram_tensor("dbg_comb", [C, 16 * C], bf16, kind="ExternalOutput").ap(),
            "demP": nc.dram_tensor("dbg_demP", [C, 1], f32, kind="ExternalOutput").ap(),
        }

    with tile.TileContext(nc) as tc:
        with ExitStack() as ctx:
            _conv_kernel(ctx, tc, y, x, wbT, ll_d, idm_d, dbg)
    nc.compile()
    return nc


_CACHE = {}


def _get_nc():
    if "nc" not in _CACHE:
        _CACHE["nc"] = _build()
    return _CACHE["nc"]


def _make_in_maps(x, de_mod, Wb, lora_up, lora_down, bias):
    x = np.asarray(x, dtype=np.float32)
    de_mod = np.asarray(de_mod, dtype=np.float32)
    Wb = np.asarray(Wb, dtype=np.float32)
    lora_up = np.asarray(lora_up, dtype=np.float32)
    lora_down = np.asarray(lora_down, dtype=np.float32)
    bias = np.asarray(bias, dtype=np.float32).reshape(C)
    # layout-only host prep: pad + cast x, transpose/replicate weights
    xp = np.zeros((B, C, HP, WP), dtype=ml_dtypes.bfloat16)
    xp[:, :, 1 : 1 + H, 1 : 1 + W] = x.astype(ml_dtypes.bfloat16)
    wbT = np.ascontiguousarray(
        Wb.transpose(1, 2, 3, 0).reshape(C, 9 * C)
    ).astype(ml_dtypes.bfloat16)
    luT = np.ascontiguousarray((SCALING * lora_up).T).astype(ml_dtypes.bfloat16)
    ldT = np.ascontiguousarray(
        lora_down.transpose(0, 2, 3, 1).reshape(RANK, 9 * C)
    ).astype(ml_dtypes.bfloat16)
    ll = np.concatenate([ldT, luT], axis=1)      # [32, 1280] bf16
    eye_f = np.eye(128, dtype=np.float32)
    eye_b = np.eye(128, dtype=ml_dtypes.bfloat16)
    eye_b_as_f = np.ascontiguousarray(eye_b).view(np.float32)  # [128, 64]
    in_maps = []
    for b in range(NCORES):
        idm = np.concatenate(
            [eye_f, np.stack([de_mod[b], bias], axis=1), eye_b_as_f], axis=1
        ).astype(np.float32)
        in_maps.append(
            {
                "x": np.ascontiguousarray(xp[b]),
                "WbT": wbT,
                "LL": ll,
                "IDm": np.ascontiguousarray(idm),
            }
        )
    return in_maps


def run(inputs, trace=False, trace_kwargs=None):
    nc = _get_nc()
    in_maps = _make_in_maps(**inputs)
    res = run_bass_kernel_spmd(
        nc,
        in_maps,
        core_ids=list(range(NCORES)),
        trace=trace,
        **(trace_kwargs or {}),
    )
    y = np.stack(
        [res.results[b]["y"].astype(np.float32) for b in range(NCORES)], axis=0
    )
    return y, res


def kernel(**inputs):
    y, _ = run(inputs)
    return y


# revision 18
# speedup vs baseline: 1.0822x; 1.0128x over previous
"""Trainium2 Bass kernel for nn_NeuronS3DiffUpsample2D.

Reference computation (per sample b):
    up   = nearest-2x-upsample(x[b])                       # [C, 320, 320]
    w    = Wb + 0.25 * einsum('or,rikl->oikl', lora_up, lora_down)
    w_b  = w * de_mod[b, None, :, None, None]              # modulate input chans
    dem  = rsqrt(sum_{i,k,l} w_b^2 + eps)                  # per output chan
    y[b] = conv2d(up, w_b, SAME) * dem + bias

Key algebraic transform: a 3x3 SAME conv on a 2x nearest-upsampled image
decomposes into 4 output phases (di, dj in {0,1}), each a 2x2 conv on the
ORIGINAL 160x160 input:
    y[2i+di, 2j+dj] = sum_{a,b in {0,1}} K[di,dj,a,b] @ x[i+a+di-1, j+b+dj-1]
where the 16 [O, I] matrices K are sums of 1/2/4 of the 9 taps of w.
This is 4/9 of the naive FLOPs and never materializes the upsampled image.
The demod scale is per output channel and conv is linear in w, so the conv
OUTPUT is scaled by dem[o] at PSUM eviction, fused with the bias add.

All tensors ride bf16 (rel-err budget is 2e-2; bf16 end-to-end lands ~4e-3):
x is host-padded to [C,162,162] bf16 so every band DMA is one contiguous
descriptor per partition (no per-row 640B descriptors, no on-device border
zeroing), matmuls are bf16 (1 cycle/row at any free size - also kills the
f32r N<256 4x penalty on the R=1 tail block), and y is written bf16 and
upcast on the host.  Total HBM traffic drops from 66 MB to ~35 MB per core,
taking DMA well below the PE's ~173 us of matmul streaming.

The demod reduction runs in a second [o, (t,i)] weight layout so the
partition-axis sum becomes a free-axis reduce: 3 small early matmuls build
delta in [o,.] form, then gpsimd/DVE/ACT finish rsqrt off the PE's program
order (the baseline's ones-matmul made the PE wait ~10us on a DVE chain).
Dummy "warm" matmuls keep the PE busy through the weight stage so the
2.4 GHz pstate ramp completes before the main conv stream starts.

Sharding: data-parallel over batch B=8 across 8 NeuronCores; each core
builds its own per-sample weights locally.  Host-side work is layout only:
padding, per-sample slicing/replication, transposes, dtype casts.
"""

import os
import sys
import numpy as np
from contextlib import ExitStack

_NOWARM = bool(int(os.environ.get("K_NOWARM", "0")))
_NOGPS = bool(int(os.environ.get("K_NOGPS", "0")))
_NODEMO = bool(int(os.environ.get("K_NODEMO", "0")))   # demod=1.0 (wrong numerics, hang-test only)
_F32OUT = bool(int(os.environ.get("K_F32OUT", "0")))
_NOACTMUL = bool(int(os.environ.get("K_NOACTMUL", "0")))
_DEBUG = bool(int(os.environ.get("K_DEBUG", "0")))

try:
    import concourse.bass as bass
except ImportError:  # grading env without the axon PYTHONPATH
    sys.path.insert(0, "/opt/trn_rl_repo")
    import concourse.bass as bass
import ml_dtypes
import concourse.tile as tile
from concourse import bacc, mybir
from concourse.bass_utils import run_bass_kernel_spmd

B, C, H, W = 8, 128, 160, 160
RANK = 32
SCALING = 0.25
EPS = 1e-8
HP, WP = H + 2, W + 2      # host-padded input (zero border baked in)
R_BLK = 3                  # x-rows per matmul block -> N = 3*160 = 480 <= 512
# x-row band split: band 0 tiny so the first matmuls gate only on ~0.3 MB.
BAND_ROWS = [3, 12, 27, 27, 27, 27, 27, 10]
BAND_START = [0, 3, 15, 42, 69, 96, 123, 150]
NBANDS = len(BAND_ROWS)
NBLOCKS = 54               # 53 blocks of 3 rows + 1 tail row
N_WARM = 8                 # PE pstate prewarm matmuls
NCORES = 8

f32 = mybir.dt.float32
bf16 = mybir.dt.bfloat16


def _conv_kernel(ctx, tc, y, x, wbT, ll_d, idm_d, dbg=None):
    nc = tc.nc
    AF = mybir.ActivationFunctionType
    ALU = mybir.AluOpType
    AX = mybir.AxisListType

    const = ctx.enter_context(tc.tile_pool(name="const", bufs=1))
    bands = ctx.enter_context(tc.tile_pool(name="bands", bufs=4))

    comb = const.tile([128, 16, C], bf16)        # 16 combined taps, [i, slot, o]
    demP = const.tile([128, 1], f32)             # rsqrt demod, per output chan
    biasT = const.tile([128, 1], f32)            # bias, copied out of IDm

    # ---- input bands: host-padded rows [S, S+rows+2) land contiguously; one
    # descriptor per partition.  Bands 0/1 ride the sync HWDGE ring (boots
    # first), the rest the gpsimd SWDGE ring.
    band_tiles = []
    band_dmas = []
    for bb in range(NBANDS):
        n = BAND_ROWS[bb] + 2
        bt = bands.tile([128, n, WP], bf16, tag="band", name=f"band{bb}")
        band_tiles.append(bt)
        band_dmas.append((bt[:, 0:n, :], x[:, BAND_START[bb] : BAND_START[bb] + n, :]))

    with tc.tile_pool(name="wtmp", bufs=1) as wtmp, tc.tile_pool(
        name="wpsum", bufs=1, space="PSUM"
    ) as wpsum:
        LL = wtmp.tile([RANK, 1280], bf16)       # lora_down^T (t,i) | 0.25*lora_up^T
        WbTS = wtmp.tile([128, 9, C], bf16)      # Wb^T: [i, t, o]
        IDm = wtmp.tile([128, 66], f32)          # de_mod,bias | bf16 eye
        LD9 = LL[:, 0:1152].rearrange("p (t c) -> p t c", c=C)
        LUTn = LL[:, 1152:1280]
        dmb = IDm[:, 0:2]
        identb = IDm[:, 2:66].bitcast(bf16)

        # sync ring: few DMA instructions, few descriptors, urgent first.
        # The DMA engines ramp for ~10us after boot (~74 GB/s aggregate), so
        # the weight-chain gate tensors go first and are as small as possible.
        nc.sync.dma_start(LL[:], ll_d[:])
        nc.sync.dma_start(IDm[:], idm_d[:])
        nc.sync.dma_start(WbTS[:], wbT[:])
        nc.sync.dma_start(*band_dmas[0])
        nc.sync.dma_start(*band_dmas[1])
        # gpsimd SWDGE: band 2 fires now; bands 3+ recycle band buffers, so
        # their WAR-gated dma_starts are emitted AFTER gpsimd's compute ops
        # (a dma_start parked on a WAR semaphore would otherwise block the
        # comb-slot builds the PE needs to make progress -> deadlock).
        nc.gpsimd.dma_start(*band_dmas[2])

        # ---- PE early: prewarm first (only needs LL; keeps the PE busy so
        # the pstate ramp to 2.4 GHz completes during the weight stage), then
        # per tap an adjacent lora+identity pair accumulating
        #   deltaP[i,t,o] = sum_r down[r,i,t]*0.25*up[o,r] + Wb^T[i,t,o]
        # (folds the base weight in on the PE; wm3 is then a single de_mod
        # multiply on DVE).  The pairs must be adjacent: interleaving other
        # matmuls inside an open accumulation group drops the first write.
        ld9f = LL[:, 0:1152]
        if not _NOWARM:
            warmP = wpsum.tile([128, 384], f32)
            for _ in range(N_WARM):
                nc.tensor.matmul(
                    warmP[:], LUTn[:], ld9f[:, 0:384], start=True, stop=True
                )
        deltaP = wpsum.tile([128, 9, C], f32)
        for t in range(9):
            nc.tensor.matmul(
                deltaP[:, t, :], LD9[:, t, :], LUTn[:], start=True, stop=False
            )
            nc.tensor.matmul(
                deltaP[:, t, :], identb[:], WbTS[:, t, :], start=False, stop=True
            )

        # bias is read during evictions long after IDm's SBUF may recycle
        nc.scalar.copy(biasT[:], dmb[:, 1:2])

        # ---- modulated weights wm3[i,t,o] = (Wb^T + delta) * de_mod[i].
        # gpsimd cannot read PSUM, so every PSUM-consuming op lives on DVE;
        # gpsimd gets the SBUF-only combine work.
        gps = nc.vector if _NOGPS else nc.gpsimd
        wm3 = wtmp.tile([128, 9, C], bf16)
        nc.vector.tensor_scalar_mul(wm3[:], deltaP[:], dmb[:, 0:1])

        # 16 combined tap matrices.  Row combos over ki (t = 3*ki + kj):
        #   (di=0, a=0): ki0        (di=0, a=1): ki1+ki2
        #   (di=1, a=0): ki0+ki1    (di=1, a=1): ki2
        # and the same pattern over kj for (dj, b).
        R01 = wtmp.tile([128, 3, C], bf16)
        nc.vector.tensor_add(R01[:], wm3[:, 3:6, :], wm3[:, 6:9, :])
        R10 = wtmp.tile([128, 3, C], bf16)
        gps.tensor_add(R10[:], wm3[:, 0:3, :], wm3[:, 3:6, :])
        rowsrc = {
            (0, 0): wm3[:, 0:3, :],
            (0, 1): R01[:],
            (1, 0): R10[:],
            (1, 1): wm3[:, 6:9, :],
        }

        # comb slot layout: slot = 8*di + 2*a + 4*dj + b (bf16 on write).
        # DVE builds phases 0/2, gpsimd phases 1/3, in PE-use order.
        def build_slot(eng, p, q):
            di, dj = p >> 1, p & 1
            a, b = q >> 1, q & 1
            S = rowsrc[(di, a)]
            dst = comb[:, 8 * di + 2 * a + 4 * dj + b, :]
            if dj == 0 and b == 0:
                eng.tensor_copy(dst, S[:, 0, :])
            elif dj == 1 and b == 1:
                eng.tensor_copy(dst, S[:, 2, :])
            elif dj == 0:
                eng.tensor_add(dst, S[:, 1, :], S[:, 2, :])
            else:
                eng.tensor_add(dst, S[:, 0, :], S[:, 1, :])

        for q in range(4):
            build_slot(nc.vector, 0, q)
        for q in range(4):
            build_slot(gps, 1, q)
        for q in range(4):
            build_slot(gps, 3, q)
        for q in range(4):
            build_slot(nc.vector, 2, q)

        # ---- demod: PE-transpose wm3 into [o, (t,i)] PSUM, then one ACT
        # Square pass whose accum_out IS the free-axis sum.  No extra DMAs,
        # no partition reduction, and the only PE cost is ~1us of transposes
        # emitted after the prewarm (wm3 is ready by then).
        if not _NODEMO:
            wmOT = wpsum.tile([128, 9, C], bf16)
            for t in range(9):
                nc.tensor.transpose(wmOT[:, t, :], wm3[:, t, :], identb[:])
        if _NODEMO:
            nc.vector.memset(demP[:], 1.0)
        else:
            sqscr = wtmp.tile([128, 9, C], f32)
            s2o = wtmp.tile([128, 1], f32)
            nc.scalar.activation(
                sqscr[:], wmOT[:], AF.Square, accum_out=s2o[:]
            )
            t1 = wtmp.tile([128, 1], f32)
            nc.vector.tensor_scalar_add(t1[:], s2o[:], EPS)
            t2 = wtmp.tile([128, 1], f32)
            nc.scalar.sqrt(t2[:], t1[:])
            nc.vector.reciprocal(demP[:], t2[:])

        if dbg is not None:
            nc.sync.dma_start(dbg["wm3"][:], wm3[:])
            nc.sync.dma_start(dbg["comb"][:], comb.rearrange("p s c -> p (s c)"))
            nc.sync.dma_start(dbg["demP"][:], demP[:])

        # now it is safe to park gpsimd on the recycled-band WAR semaphores
        for bb in range(3, NBANDS):
            nc.gpsimd.dma_start(*band_dmas[bb])

    # ---- main conv loop ----
    mpsum = ctx.enter_context(tc.tile_pool(name="mpsum", bufs=8, space="PSUM"))
    opool = ctx.enter_context(tc.tile_pool(name="obuf", bufs=3))

    # blocks of 3 x-rows; pairs of blocks share one output staging tile so
    # the sync sequencer issues half as many (128-descriptor) output DMAs.
    blk_band = []
    for g in range(NBLOCKS):
        i0 = g * R_BLK
        bb = next(
            b for b in range(NBANDS)
            if BAND_START[b] <= i0 and i0 + min(R_BLK, H - i0) <= BAND_START[b] + BAND_ROWS[b]
        )
        blk_band.append(bb)

    ob = None
    for g in range(NBLOCKS):
        i0 = g * R_BLK
        R = min(R_BLK, H - i0)
        bb = blk_band[g]
        bt = band_tiles[bb]
        ph = []
        for p in range(4):
            di, dj = p >> 1, p & 1
            pt = mpsum.tile([128, R * W], f32, tag="ph", name=f"ph{p}_{i0}")
            for q in range(4):
                a, b = q >> 1, q & 1
                tr = i0 + a + di - BAND_START[bb]     # padded-tile row
                co = b + dj                           # padded-tile col
                rhs = bt[:, tr : tr + R, co : co + W]
                slot = 8 * di + 2 * a + 4 * dj + b
                nc.tensor.matmul(
                    pt[:], comb[:, slot, :], rhs,
                    start=(q == 0), stop=(q == 3),
                )
            ph.append(pt)

        if g % 2 == 0:
            RR = R + (min(R_BLK, H - i0 - R) if g + 1 < NBLOCKS else 0)
            ob = opool.tile(
                [128, RR, 2, 2 * W], f32 if _F32OUT else bf16, tag="ob", name=f"ob_{i0}"
            )
            lr0 = 0
        else:
            lr0 = R_BLK
        obv = ob.rearrange("p r d (j two) -> p r d two j", two=2)
        # interleave phases into full output rows; scale by demod, add bias
        for p in range(4):
            di, dj = p >> 1, p & 1
            dst = obv[:, lr0 : lr0 + R, di, dj, :]
            srcv = ph[p].rearrange("p (r j) -> p r j", r=R)
            if dj == 0:
                nc.vector.tensor_scalar(
                    dst, srcv, demP[:, 0:1], biasT[:, 0:1],
                    op0=ALU.mult, op1=ALU.add,
                )
            else:
                nc.scalar.activation(
                    dst, srcv, AF.Identity, bias=biasT[:, 0:1], scale=demP[:, 0:1]
                )
        if g % 2 == 1 or g == NBLOCKS - 1:
            g0 = g - (g % 2)
            y0 = 2 * g0 * R_BLK
            nrows = 2 * ob.shape[1]
            nc.sync.dma_start(y[:, y0 : y0 + nrows, :], ob[:])


def _build():
    nc = bacc.Bacc(
        "TRN2",
        target_bir_lowering=False,
        debug=False,
        enable_asserts=False,
        num_devices=NCORES,
    )
    x = nc.dram_tensor("x", [C, HP, WP], bf16, kind="ExternalInput").ap()
    wbT = nc.dram_tensor("WbT", [C, 9 * C], bf16, kind="ExternalInput").ap()
    ll_d = nc.dram_tensor("LL", [RANK, 1280], bf16, kind="ExternalInput").ap()
    idm_d = nc.dram_tensor("IDm", [128, 66], f32, kind="ExternalInput").ap()
    y = nc.dram_tensor(
        "y", [C, 2 * H, 2 * W], f32 if _F32OUT else bf16, kind="ExternalOutput"
    ).ap()
    dbg = None
    if _DEBUG:
        dbg = {
            "wm3": nc.dram_tensor("dbg_wm3", [C, 9 * C], f32, kind="ExternalOutput").ap(),
            "comb": nc.dram_tensor("dbg_comb", [C, 16 * C], bf16, kind="ExternalOutput").ap(),
            "demP": nc.dram_tensor("dbg_demP", [C, 1], f32, kind="ExternalOutput").ap(),
        }

    with tile.TileContext(nc) as tc:
        with ExitStack() as ctx:
            _conv_kernel(ctx, tc, y, x, wbT, ll_d, idm_d, dbg)
    nc.compile()
    return nc


_CACHE = {}


def _get_nc():
    if "nc" not in _CACHE:
        _CACHE["nc"] = _build()
    return _CACHE["nc"]


def _make_in_maps(x, de_mod, Wb, lora_up, lora_down, bias):
    x = np.asarray(x, dtype=np.float32)
    de_mod = np.asarray(de_mod, dtype=np.float32)
    Wb = np.asarray(Wb, dtype=np.float32)
    lora_up = np.asarray(lora_up, dtype=np.float32)
    lora_down = np.asarray(lora_down, dtype=np.float32)
    bias = np.asarray(bias, dtype=np.float32).reshape(C)
    # layout-only host prep: pad + cast x, transpose/replicate weights
    xp = np.zeros((B, C, HP, WP), dtype=ml_dtypes.bfloat16)
    xp[:, :, 1 : 1 + H, 1 : 1 + W] = x.astype(ml_dtypes.bfloat16)
    wbT = np.ascontiguousarray(
        Wb.transpose(1, 2, 3, 0).reshape(C, 9 * C)
    ).astype(ml_dtypes.bfloat16)
    luT = np.ascontiguousarray((SCALING * lora_up).T).astype(ml_dtypes.bfloat16)
    ldT = np.ascontiguousarray(
        lora_down.transpose(0, 2, 3, 1).reshape(RANK, 9 * C)
    ).astype(ml_dtypes.bfloat16)
    ll = np.concatenate([ldT, luT], axis=1)      # [32, 1280] bf16
    eye_b = np.eye(128, dtype=ml_dtypes.bfloat16)
    eye_b_as_f = np.ascontiguousarray(eye_b).view(np.float32)  # [128, 64]
    in_maps = []
    for b in range(NCORES):
        idm = np.concatenate(
            [np.stack([de_mod[b], bias], axis=1), eye_b_as_f], axis=1
        ).astype(np.float32)
        in_maps.append(
            {
                "x": np.ascontiguousarray(xp[b]),
                "WbT": wbT,
                "LL": ll,
                "IDm": np.ascontiguousarray(idm),
            }
        )
    return in_maps


def run(inputs, trace=False, trace_kwargs=None):
    nc = _get_nc()
    in_maps = _make_in_maps(**inputs)
    res = run_bass_kernel_spmd(
        nc,
        in_maps,
        core_ids=list(range(NCORES)),
        trace=trace,
        **(trace_kwargs or {}),
    )
    y = np.stack(
        [res.results[b]["y"].astype(np.float32) for b in range(NCORES)], axis=0
    )
    return y, res


def kernel(**inputs):
    y, _ = run(inputs)
    return y


# revision 19
# speedup vs baseline: 1.1180x; 1.0331x over previous
"""Trainium2 Bass kernel for nn_NeuronS3DiffUpsample2D.

Reference computation (per sample b):
    up   = nearest-2x-upsample(x[b])                       # [C, 320, 320]
    w    = Wb + 0.25 * einsum('or,rikl->oikl', lora_up, lora_down)
    w_b  = w * de_mod[b, None, :, None, None]              # modulate input chans
    dem  = rsqrt(sum_{i,k,l} w_b^2 + eps)                  # per output chan
    y[b] = conv2d(up, w_b, SAME) * dem + bias

Key algebraic transform: a 3x3 SAME conv on a 2x nearest-upsampled image
decomposes into 4 output phases (di, dj in {0,1}), each a 2x2 conv on the
ORIGINAL 160x160 input:
    y[2i+di, 2j+dj] = sum_{a,b in {0,1}} K[di,dj,a,b] @ x[i+a+di-1, j+b+dj-1]
where the 16 [O, I] matrices K are sums of 1/2/4 of the 9 taps of w.
This is 4/9 of the naive FLOPs and never materializes the upsampled image.
The demod scale is per output channel and conv is linear in w, so the conv
OUTPUT is scaled by dem[o] at PSUM eviction, fused with the bias add.

All tensors ride bf16 (rel-err budget is 2e-2; bf16 end-to-end lands ~4e-3):
x is host-padded to [C,162,162] bf16 so every band DMA is one contiguous
descriptor per partition (no per-row 640B descriptors, no on-device border
zeroing), matmuls are bf16 (1 cycle/row at any free size - also kills the
f32r N<256 4x penalty on the R=1 tail block), and y is written bf16 and
upcast on the host.  Total HBM traffic drops from 66 MB to ~35 MB per core,
taking DMA well below the PE's ~173 us of matmul streaming.

The demod reduction runs in a second [o, (t,i)] weight layout so the
partition-axis sum becomes a free-axis reduce: 3 small early matmuls build
delta in [o,.] form, then gpsimd/DVE/ACT finish rsqrt off the PE's program
order (the baseline's ones-matmul made the PE wait ~10us on a DVE chain).
Dummy "warm" matmuls keep the PE busy through the weight stage so the
2.4 GHz pstate ramp completes before the main conv stream starts.

Sharding: data-parallel over batch B=8 across 8 NeuronCores; each core
builds its own per-sample weights locally.  Host-side work is layout only:
padding, per-sample slicing/replication, transposes, dtype casts.
"""

import os
import sys
import numpy as np
from contextlib import ExitStack

_NOWARM = bool(int(os.environ.get("K_NOWARM", "0")))
_NOGPS = bool(int(os.environ.get("K_NOGPS", "0")))
_NODEMO = bool(int(os.environ.get("K_NODEMO", "0")))   # demod=1.0 (wrong numerics, hang-test only)
_F32OUT = bool(int(os.environ.get("K_F32OUT", "0")))
_NOACTMUL = bool(int(os.environ.get("K_NOACTMUL", "0")))
_DEBUG = bool(int(os.environ.get("K_DEBUG", "0")))

try:
    import concourse.bass as bass
except ImportError:  # grading env without the axon PYTHONPATH
    sys.path.insert(0, "/opt/trn_rl_repo")
    import concourse.bass as bass
import ml_dtypes
import concourse.tile as tile
from concourse import bacc, mybir
from concourse.bass_utils import run_bass_kernel_spmd

B, C, H, W = 8, 128, 160, 160
RANK = 32
SCALING = 0.25
EPS = 1e-8
HP, WP = H + 2, W + 2      # host-padded input (zero border baked in)
R_BLK = 3                  # x-rows per matmul block -> N = 3*160 = 480 <= 512
# x-row band split: band 0 tiny so the first matmuls gate only on ~0.3 MB.
BAND_ROWS = [3, 12, 27, 27, 27, 27, 27, 10]
BAND_START = [0, 3, 15, 42, 69, 96, 123, 150]
NBANDS = len(BAND_ROWS)
NBLOCKS = 54               # 53 blocks of 3 rows + 1 tail row
N_WARM = 4                 # PE pstate prewarm matmuls
NCORES = 8

f32 = mybir.dt.float32
bf16 = mybir.dt.bfloat16


def _conv_kernel(ctx, tc, y, x, wbT, ll_d, idm_d, dbg=None):
    nc = tc.nc
    AF = mybir.ActivationFunctionType
    ALU = mybir.AluOpType
    AX = mybir.AxisListType

    const = ctx.enter_context(tc.tile_pool(name="const", bufs=1))
    bands = ctx.enter_context(tc.tile_pool(name="bands", bufs=4))

    comb = const.tile([128, 16, C], bf16)        # 16 combined taps, [i, slot, o]
    demP = const.tile([128, 1], f32)             # rsqrt demod, per output chan
    biasT = const.tile([128, 1], f32)            # bias, copied out of IDm

    # ---- input bands: host-padded rows [S, S+rows+2) land contiguously; one
    # descriptor per partition.  Bands 0/1 ride the sync HWDGE ring (boots
    # first), the rest the gpsimd SWDGE ring.
    band_tiles = []
    band_dmas = []
    for bb in range(NBANDS):
        n = BAND_ROWS[bb] + 2
        bt = bands.tile([128, n, WP], bf16, tag="band", name=f"band{bb}")
        band_tiles.append(bt)
        band_dmas.append((bt[:, 0:n, :], x[:, BAND_START[bb] : BAND_START[bb] + n, :]))

    with tc.tile_pool(name="wtmp", bufs=1) as wtmp, tc.tile_pool(
        name="wpsum", bufs=1, space="PSUM"
    ) as wpsum:
        LL = wtmp.tile([RANK, 1280], bf16)       # lora_down^T (t,i) | 0.25*lora_up^T
        WbTS = wtmp.tile([128, 9, C], bf16)      # Wb^T: [i, t, o]
        IDm = wtmp.tile([128, 66], f32)          # de_mod,bias | bf16 eye
        LD9 = LL[:, 0:1152].rearrange("p (t c) -> p t c", c=C)
        LUTn = LL[:, 1152:1280]
        dmb = IDm[:, 0:2]
        identb = IDm[:, 2:66].bitcast(bf16)

        # The SWDGE (gpsimd) queue hits full rate (~380 GB/s) immediately,
        # while the HWDGE (sync) queue crawls at <70 GB/s for its first ~8us.
        # So ALL startup tensors ride SWDGE, gate tensors first; sync gets a
        # small primer so its ramp is done before the first output eviction.
        nc.gpsimd.dma_start(LL[:], ll_d[:])
        nc.gpsimd.dma_start(IDm[:], idm_d[:])
        nc.gpsimd.dma_start(WbTS[:], wbT[:])
        for bb in range(4):
            nc.gpsimd.dma_start(*band_dmas[bb])
        LLscratch = wtmp.tile([RANK, 1280], bf16, name="llscratch")
        nc.sync.dma_start(LLscratch[:], ll_d[:])

        # ---- PE early: prewarm first (only needs LL; keeps the PE busy so
        # the pstate ramp to 2.4 GHz completes during the weight stage), then
        # per tap an adjacent lora+identity pair accumulating
        #   deltaP[i,t,o] = sum_r down[r,i,t]*0.25*up[o,r] + Wb^T[i,t,o]
        # (folds the base weight in on the PE; wm3 is then a single de_mod
        # multiply on DVE).  The pairs must be adjacent: interleaving other
        # matmuls inside an open accumulation group drops the first write.
        ld9f = LL[:, 0:1152]
        if not _NOWARM:
            warmP = wpsum.tile([128, 384], f32)
            for _ in range(N_WARM):
                nc.tensor.matmul(
                    warmP[:], LUTn[:], ld9f[:, 0:384], start=True, stop=True
                )
        deltaP = wpsum.tile([128, 9, C], f32)
        for t in range(9):
            nc.tensor.matmul(
                deltaP[:, t, :], LD9[:, t, :], LUTn[:], start=True, stop=False
            )
            nc.tensor.matmul(
                deltaP[:, t, :], identb[:], WbTS[:, t, :], start=False, stop=True
            )

        # bias is read during evictions long after IDm's SBUF may recycle
        nc.scalar.copy(biasT[:], dmb[:, 1:2])

        # ---- modulated weights wm3[i,t,o] = (Wb^T + delta) * de_mod[i].
        # gpsimd cannot read PSUM, so every PSUM-consuming op lives on DVE;
        # gpsimd gets the SBUF-only combine work.
        gps = nc.vector if _NOGPS else nc.gpsimd
        wm3 = wtmp.tile([128, 9, C], bf16)
        nc.vector.tensor_scalar_mul(wm3[:], deltaP[:], dmb[:, 0:1])

        # 16 combined tap matrices.  Row combos over ki (t = 3*ki + kj):
        #   (di=0, a=0): ki0        (di=0, a=1): ki1+ki2
        #   (di=1, a=0): ki0+ki1    (di=1, a=1): ki2
        # and the same pattern over kj for (dj, b).
        R01 = wtmp.tile([128, 3, C], bf16)
        nc.vector.tensor_add(R01[:], wm3[:, 3:6, :], wm3[:, 6:9, :])
        R10 = wtmp.tile([128, 3, C], bf16)
        gps.tensor_add(R10[:], wm3[:, 0:3, :], wm3[:, 3:6, :])
        rowsrc = {
            (0, 0): wm3[:, 0:3, :],
            (0, 1): R01[:],
            (1, 0): R10[:],
            (1, 1): wm3[:, 6:9, :],
        }

        # comb slot layout: slot = 8*di + 2*a + 4*dj + b (bf16 on write).
        # DVE builds phases 0/2, gpsimd phases 1/3, in PE-use order.
        def build_slot(eng, p, q):
            di, dj = p >> 1, p & 1
            a, b = q >> 1, q & 1
            S = rowsrc[(di, a)]
            dst = comb[:, 8 * di + 2 * a + 4 * dj + b, :]
            if dj == 0 and b == 0:
                eng.tensor_copy(dst, S[:, 0, :])
            elif dj == 1 and b == 1:
                eng.tensor_copy(dst, S[:, 2, :])
            elif dj == 0:
                eng.tensor_add(dst, S[:, 1, :], S[:, 2, :])
            else:
                eng.tensor_add(dst, S[:, 0, :], S[:, 1, :])

        for q in range(4):
            build_slot(nc.vector, 0, q)
        for q in range(4):
            build_slot(gps, 1, q)
        for q in range(4):
            build_slot(gps, 3, q)
        for q in range(4):
            build_slot(nc.vector, 2, q)

        # ---- demod: PE-transpose wm3 into [o, (t,i)] PSUM, then one ACT
        # Square pass whose accum_out IS the free-axis sum.  No extra DMAs,
        # no partition reduction, and the only PE cost is ~1us of transposes
        # emitted after the prewarm (wm3 is ready by then).
        if not _NODEMO:
            wmOT = wpsum.tile([128, 9, C], bf16)
            for t in range(9):
                nc.tensor.transpose(wmOT[:, t, :], wm3[:, t, :], identb[:])
        if _NODEMO:
            nc.vector.memset(demP[:], 1.0)
        else:
            sqscr = wtmp.tile([128, 9, C], f32)
            s2o = wtmp.tile([128, 1], f32)
            nc.scalar.activation(
                sqscr[:], wmOT[:], AF.Square, accum_out=s2o[:]
            )
            t1 = wtmp.tile([128, 1], f32)
            nc.vector.tensor_scalar_add(t1[:], s2o[:], EPS)
            t2 = wtmp.tile([128, 1], f32)
            nc.scalar.sqrt(t2[:], t1[:])
            nc.vector.reciprocal(demP[:], t2[:])

        if dbg is not None:
            nc.sync.dma_start(dbg["wm3"][:], wm3[:])
            nc.sync.dma_start(dbg["comb"][:], comb.rearrange("p s c -> p (s c)"))
            nc.sync.dma_start(dbg["demP"][:], demP[:])

        # now it is safe to park gpsimd on the recycled-band WAR semaphores
        # (bands 4+ reuse band 0-3 buffers)
        for bb in range(4, NBANDS):
            nc.gpsimd.dma_start(*band_dmas[bb])

    # ---- main conv loop ----
    mpsum = ctx.enter_context(tc.tile_pool(name="mpsum", bufs=8, space="PSUM"))
    opool = ctx.enter_context(tc.tile_pool(name="obuf", bufs=3))

    # blocks of 3 x-rows; pairs of blocks share one output staging tile so
    # the sync sequencer issues half as many (128-descriptor) output DMAs.
    blk_band = []
    for g in range(NBLOCKS):
        i0 = g * R_BLK
        bb = next(
            b for b in range(NBANDS)
            if BAND_START[b] <= i0 and i0 + min(R_BLK, H - i0) <= BAND_START[b] + BAND_ROWS[b]
        )
        blk_band.append(bb)

    ob = None
    for g in range(NBLOCKS):
        i0 = g * R_BLK
        R = min(R_BLK, H - i0)
        bb = blk_band[g]
        bt = band_tiles[bb]
        ph = []
        for p in range(4):
            di, dj = p >> 1, p & 1
            pt = mpsum.tile([128, R * W], f32, tag="ph", name=f"ph{p}_{i0}")
            for q in range(4):
                a, b = q >> 1, q & 1
                tr = i0 + a + di - BAND_START[bb]     # padded-tile row
                co = b + dj                           # padded-tile col
                rhs = bt[:, tr : tr + R, co : co + W]
                slot = 8 * di + 2 * a + 4 * dj + b
                nc.tensor.matmul(
                    pt[:], comb[:, slot, :], rhs,
                    start=(q == 0), stop=(q == 3),
                )
            ph.append(pt)

        if g % 2 == 0:
            RR = R + (min(R_BLK, H - i0 - R) if g + 1 < NBLOCKS else 0)
            ob = opool.tile(
                [128, RR, 2, 2 * W], f32 if _F32OUT else bf16, tag="ob", name=f"ob_{i0}"
            )
            lr0 = 0
        else:
            lr0 = R_BLK
        obv = ob.rearrange("p r d (j two) -> p r d two j", two=2)
        # interleave phases into full output rows; scale by demod, add bias
        for p in range(4):
            di, dj = p >> 1, p & 1
            dst = obv[:, lr0 : lr0 + R, di, dj, :]
            srcv = ph[p].rearrange("p (r j) -> p r j", r=R)
            if dj == 0:
                nc.vector.tensor_scalar(
                    dst, srcv, demP[:, 0:1], biasT[:, 0:1],
                    op0=ALU.mult, op1=ALU.add,
                )
            else:
                nc.scalar.activation(
                    dst, srcv, AF.Identity, bias=biasT[:, 0:1], scale=demP[:, 0:1]
                )
        if g % 2 == 1 or g == NBLOCKS - 1:
            g0 = g - (g % 2)
            y0 = 2 * g0 * R_BLK
            nrows = 2 * ob.shape[1]
            nc.sync.dma_start(y[:, y0 : y0 + nrows, :], ob[:])


def _build():
    nc = bacc.Bacc(
        "TRN2",
        target_bir_lowering=False,
        debug=False,
        enable_asserts=False,
        num_devices=NCORES,
    )
    x = nc.dram_tensor("x", [C, HP, WP], bf16, kind="ExternalInput").ap()
    wbT = nc.dram_tensor("WbT", [C, 9 * C], bf16, kind="ExternalInput").ap()
    ll_d = nc.dram_tensor("LL", [RANK, 1280], bf16, kind="ExternalInput").ap()
    idm_d = nc.dram_tensor("IDm", [128, 66], f32, kind="ExternalInput").ap()
    y = nc.dram_tensor(
        "y", [C, 2 * H, 2 * W], f32 if _F32OUT else bf16, kind="ExternalOutput"
    ).ap()
    dbg = None
    if _DEBUG:
        dbg = {
            "wm3": nc.dram_tensor("dbg_wm3", [C, 9 * C], f32, kind="ExternalOutput").ap(),
            "comb": nc.dram_tensor("dbg_comb", [C, 16 * C], bf16, kind="ExternalOutput").ap(),
            "demP": nc.dram_tensor("dbg_demP", [C, 1], f32, kind="ExternalOutput").ap(),
        }

    with tile.TileContext(nc) as tc:
        with ExitStack() as ctx:
            _conv_kernel(ctx, tc, y, x, wbT, ll_d, idm_d, dbg)
    nc.compile()
    return nc


_CACHE = {}


def _get_nc():
    if "nc" not in _CACHE:
        _CACHE["nc"] = _build()
    return _CACHE["nc"]


def _make_in_maps(x, de_mod, Wb, lora_up, lora_down, bias):
    x = np.asarray(x, dtype=np.float32)
    de_mod = np.asarray(de_mod, dtype=np.float32)
    Wb = np.asarray(Wb, dtype=np.float32)
    lora_up = np.asarray(lora_up, dtype=np.float32)
    lora_down = np.asarray(lora_down, dtype=np.float32)
    bias = np.asarray(bias, dtype=np.float32).reshape(C)
    # layout-only host prep: pad + cast x, transpose/replicate weights
    xp = np.zeros((B, C, HP, WP), dtype=ml_dtypes.bfloat16)
    xp[:, :, 1 : 1 + H, 1 : 1 + W] = x.astype(ml_dtypes.bfloat16)
    wbT = np.ascontiguousarray(
        Wb.transpose(1, 2, 3, 0).reshape(C, 9 * C)
    ).astype(ml_dtypes.bfloat16)
    luT = np.ascontiguousarray((SCALING * lora_up).T).astype(ml_dtypes.bfloat16)
    ldT = np.ascontiguousarray(
        lora_down.transpose(0, 2, 3, 1).reshape(RANK, 9 * C)
    ).astype(ml_dtypes.bfloat16)
    ll = np.concatenate([ldT, luT], axis=1)      # [32, 1280] bf16
    eye_b = np.eye(128, dtype=ml_dtypes.bfloat16)
    eye_b_as_f = np.ascontiguousarray(eye_b).view(np.float32)  # [128, 64]
    in_maps = []
    for b in range(NCORES):
        idm = np.concatenate(
            [np.stack([de_mod[b], bias], axis=1), eye_b_as_f], axis=1
        ).astype(np.float32)
        in_maps.append(
            {
                "x": np.ascontiguousarray(xp[b]),
                "WbT": wbT,
                "LL": ll,
                "IDm": np.ascontiguousarray(idm),
            }
        )
    return in_maps


def run(inputs, trace=False, trace_kwargs=None):
    nc = _get_nc()
    in_maps = _make_in_maps(**inputs)
    res = run_bass_kernel_spmd(
        nc,
        in_maps,
        core_ids=list(range(NCORES)),
        trace=trace,
        **(trace_kwargs or {}),
    )
    y = np.stack(
        [res.results[b]["y"].astype(np.float32) for b in range(NCORES)], axis=0
    )
    return y, res


def kernel(**inputs):
    y, _ = run(inputs)
    return y
